# revision 34
# baseline (speedup 1.0000x reference)
"""DenseCaptioner LSTM-gate kernel for 8 Trainium2 NeuronCores.

Role-split sharding (halves per-core HBM traffic vs. gate+batch-half
data parallelism):
  cores 0-3  run program VIS: visual + recurrent paths for gate g = core,
             full batch (two 128-row m-tiles)  -> partial logits [256,1024]
  cores 4-7  run program INP: input path for gate g = core-4, full batch
             -> partial logits [256,1024]
Host: logits[g] = vis_part[g] + inp_part[g] + b[g], then sigmoid/tanh gate
math and the prev_c recurrence.

Perf structure:
  - acts stationary (one [128,128] LDWEIGHTS per (k-tile, m-tile) covers
    1024 streamed weight columns), weights moving in 512-col chunks
  - level-1 weights (V1, V2, W1) ship as FP8_EXP3/e3m4: half the DMA
    bytes of the dominant streams at unchanged PE speed (fp8 without
    DoubleRow streams at bf16 rate; mixed bf16-stationary x fp8-moving
    is numerically exact in the PE's e6m3/e10m23 pipeline).  Weights are
    pre-scaled by 128 into e3m4's [0.25, 15.5] normal band; the inverse
    scale folds into the bf16 C1/W3 images host-side.  Rel err 1.57e-2
    vs the 2e-2 gate (all-bf16: 4.5e-3; deterministic inputs).
  - everything else bf16
  - weights shipped in k-tile-major SBUF-image layout [128, ktiles*H1]
    so every weight DMA is long-contiguous per partition; streamed in
    slabs with a [1, 2, 5] ramp so the PE starts ~3us after the Tile
    preamble; 6 warm-up matmuls on a memset tile fill the preamble->
    first-slab window and ramp the PE to max pstate for free
  - ring split: ALL weight slabs go on the sync (SP) HWDGE ring;
    hadamard transposes + output DMAs go on the scalar (ACT) ring; act
    chunks go through the gpsimd SWDGE path.  Tile tracks HWDGE
    completions on 8 shared semaphore lanes assigned round-robin over
    BOTH HWDGE rings, so every HWDGE DMA orders behind the 8th-previous
    one; keeping acts off those lanes makes the weight window
    self-paced and leaves the critical-path transposes ordered behind
    early slabs only.
  - PSUM allocated in [128, 512] single-bank chunks, one tag rotating
    all 8 banks; hadamard consumes chunk-wise (bounce eats the stream
    that stops first, freeing the next stage's banks early)
  - vis tail restructured: C2 runs alone right after U1U2 and its t2
    psums are copied to SBUF (freeing the banks the l3 group reuses
    without waiting on the final hadamard); C1 runs alone against the
    SBUF-resident t2 (no bounce copy); U3 opens the l3 accumulation
    (start=True, no stop), C3 continues and closes it -> the logits sum
    happens in PSUM, no vector add in the tail
  - Hadamard transposes moved off the PE onto the DMA XBAR
    (dma_start_transpose: out[p, t, b] = in[b, t*128+p], verified)
  - vis interleaves the V-chain and U-chain so the PE never waits on
    vector/XBAR between levels; last stages run m-outer so the first
    m-tile's output DMA overlaps the second's matmuls

The two programs are dispatched concurrently on disjoint device subsets
through a copy of concourse's PJRT runner that takes an explicit device
list (the stock one hardcodes jax.devices()[:n]).
"""

import numpy as np

import jax
from jax.experimental.shard_map import shard_map
from jax.sharding import Mesh, PartitionSpec

import concourse.mybir as mybir
import concourse.tile as tile
from concourse import bacc, bass2jax

B, X, V, MM, VH, H1, H2, G = 256, 12000, 4096, 1024, 1024, 1024, 1024, 4
XP = 12032  # X padded to a multiple of 128 (94 k-tiles)
N_CORES = 8
MT = 2      # m-tiles (batch 256 = 2 x 128)
NC2 = 2     # 512-col chunks per 1024 row (one PSUM bank each)

DT_NAME = "bfloat16"

# Level-1 weights ship as FP8_EXP3 (e3m4): half the DMA bytes of the
# dominant streams at unchanged PE speed (fp8 without DoubleRow runs at
# bf16 rate; only the moving operand's dtype changes).  Weights are
# pre-scaled by W8_SCALE into e3m4's [0.25, 15.5] normal band; the
# inverse scale is folded into the next-level bf16 weights (C1, W3) on
# the host.  End-to-end rel err ~1.6e-2 vs the 2e-2 gate (bf16: 4.5e-3).
W8_NAMES = {"V1", "V2", "W1"}
W8_SCALE = 128.0
W8_MAX = 15.5

_cache = {}


def _mm_dt():
    return getattr(mybir.dt, DT_NAME)


def _np_dt():
    return mybir.dt.np(_mm_dt())


def _slab_sizes(kt, ramp=False):
    """Slab progression. ramp=True starts [1, 2, 5] so the PE's first
    matmul only waits on a 256KB transfer; later-stage weights use full
    8-k-tile slabs (bigger transfers = better per-queue DMA efficiency)."""
    sizes = []
    rem = kt
    if ramp:
        for s in (1, 2, 5):
            s = min(s, rem)
            if s:
                sizes.append(s)
            rem -= s
    while rem > 0:
        sizes.append(min(8, rem))
        rem -= 8
    return sizes


def build_program(role):
    """role "vis": visual+recurrent paths; "inp": input path. Full batch."""
    dt = _mm_dt()
    f32 = mybir.dt.float32

    nc = bacc.Bacc("TRN2", target_bir_lowering=False, debug=False)

    if role == "vis":
        act_specs = {"v1T": V, "v2T": V, "mT": MM, "hT": H2}
        w_specs = {"V1": V, "V2": V, "C1": VH, "C2": MM, "C3": H1,
                   "U1": H2, "U2": MM, "U3": H1}
    else:
        act_specs = {"xT": XP, "mT": MM}
        w_specs = {"W1": XP, "W2": MM, "W3": H1}

    def wdt(name):
        return mybir.dt.float8e3 if name in W8_NAMES else dt

    acts_d = {
        name: nc.dram_tensor(name, [128, k // 128 * B], dt, kind="ExternalInput")
        for name, k in act_specs.items()
    }
    # weights in k-tile-major SBUF-image layout [128, ktiles*H1]
    wt = {
        name: nc.dram_tensor(name, [128, (k // 128) * H1], wdt(name),
                             kind="ExternalInput")
        for name, k in w_specs.items()
    }
    # partial logits leave in bf16: half the output DMA, ~0.4% of the
    # logit magnitude in rounding — negligible against the fp8 error
    out = nc.dram_tensor("out", [B, H2], dt, kind="ExternalOutput")

    with tile.TileContext(nc) as tc:
        with (
            tc.tile_pool(name="acts", bufs=1) as acts,
            tc.tile_pool(name="wstream", bufs=4) as wstream,
            tc.tile_pool(name="wstream8", bufs=6) as wstream8,
            tc.tile_pool(name="inter", bufs=1) as inter,
            tc.tile_pool(name="ps", bufs=8, space="PSUM") as ps,
        ):
            # --- pre-plan bulk DMAs. Weights ride the sync (SP) HWDGE
            # ring exclusively (self-paced by wstream buffer recycling);
            # acts ride the scalar (ACT) ring, which later also carries
            # the critical-path hadamard transposes + output DMAs with
            # nothing slow queued ahead of them. ---
            act_tiles = {}
            for name in act_specs:
                ktiles = act_specs[name] // 128
                t = acts.tile([128, ktiles * B], dt, tag=name, name=name)
                act_tiles[name] = t

            ramp_w = {"V1", "V2"} if role == "vis" else {"W1"}
            slab_plan = {}  # wname -> list of (k0, s, tile)
            for name in w_specs:
                plan, k0 = [], 0
                if role == "inp" and name == "W2":
                    sizes = [2, 6]
                else:
                    sizes = _slab_sizes(w_specs[name] // 128, ramp=name in ramp_w)
                for s in sizes:
                    if name in W8_NAMES:
                        w = wstream8.tile([128, 8 * H1], mybir.dt.float8e3,
                                          tag="w8", name="w8")
                    else:
                        w = wstream.tile([128, 8 * H1], dt, tag="w", name="w")
                    plan.append((k0, s, w))
                    k0 += s
                slab_plan[name] = plan

            def _sched(order):
                """order: ("act", name, (kt0, kt1)) or ("w", name, si).
                Weight slabs alternate across the two HWDGE rings (sync/
                scalar) for 2x cold-phase throughput; acts are few and
                coarse so the shared 8-lane completion window stays
                weight-dominated.  Hadamard transposes + output DMAs are
                emitted later on the scalar ring, ordered only behind
                early-completing slabs in the lane window."""
                wq = 0
                for item in order:
                    name = item[1]
                    if item[0] == "act":
                        t0, t1 = item[2]
                        t1 = min(t1, act_specs[name] // 128)
                        if t0 >= t1:
                            continue
                        lo, hi = t0 * B, t1 * B
                        eng = nc.sync if wq % 2 == 0 else nc.scalar
                        wq += 1
                        eng.dma_start(
                            act_tiles[name][:, lo:hi], acts_d[name].ap()[:, lo:hi]
                        )
                    else:
                        k0, s, w = slab_plan[name][item[2]]
                        eng = nc.sync if wq % 2 == 0 else nc.scalar
                        wq += 1
                        eng.dma_start(
                            w[:, :s * H1],
                            wt[name].ap()[:, k0 * H1:(k0 + s) * H1],
                        )

            act_sb = {
                name: act_tiles[name].rearrange("p (t m b) -> p t m b", m=MT, b=128)
                for name in act_specs
            }

            # warm-up: the tensor engine reaches max pstate only after
            # ~3us of continuous work, and the first real matmul can't
            # start before its slab lands (~3us after the preamble).
            # Fill that window with throwaway matmuls on a memset tile.
            warm = inter.tile([128, 512], dt, tag="warm", name="warm")
            nc.gpsimd.memset(warm[:], 0.0)
            warm_ps = ps.tile([128, 512], f32, tag="bank", name="warm_ps")
            for _ in range(6):
                nc.tensor.matmul(
                    warm_ps[:], warm[:, :128], warm[:],
                    start=True, stop=True,
                )

            def alloc_ps(tag_name, nchunks=NC2):
                """[m][c] grid of single-bank [128,512] psum tiles."""
                return [
                    [ps.tile([128, 512], f32, tag="bank",
                             name=f"{tag_name}_{mi}_{c}") for c in range(nchunks)]
                    for mi in range(MT)
                ]

            def stream(jobs, m_outer=False, open_acc=True, close_acc=True):
                """jobs: list of (psums[m][c], act_fn(k, mi)->lhsT, wname).
                Matmuls only; the slab DMAs were pre-issued in need-order.
                Round-robin across jobs so paired streams finish together.
                m_outer: single job, loop m-tiles outermost so m0's psum
                accumulation completes mid-stage and its consumers overlap
                the m1 half.
                open_acc/close_acc=False: leave the PSUM accumulation group
                open across a later stream call into the same banks."""
                if m_outer:
                    (psums, act, wname), = jobs
                    ktiles = w_specs[wname] // 128
                    for mi in range(MT):
                        for k0, s, w in slab_plan[wname]:
                            for dk in range(s):
                                k = k0 + dk
                                for c in range(NC2):
                                    nc.tensor.matmul(
                                        psums[mi][c][:],
                                        act(k, mi),
                                        w[:, dk * H1 + c * 512:
                                          dk * H1 + c * 512 + 512],
                                        start=k == 0 and open_acc,
                                        stop=k == ktiles - 1 and close_acc,
                                        skip_group_check=not (open_acc and close_acc),
                                    )
                    return
                plans = []
                for psums, act, wname in jobs:
                    plans.append({
                        "psums": psums, "act": act, "wname": wname,
                        "ktiles": w_specs[wname] // 128,
                        "si": 0,
                    })
                while any(p["si"] < len(slab_plan[p["wname"]]) for p in plans):
                    for p in plans:
                        slabs = slab_plan[p["wname"]]
                        if p["si"] >= len(slabs):
                            continue
                        k0, s, w = slabs[p["si"]]
                        # m-outer within each slab: m0's stop lands s
                        # matmul-pairs before m1's, so its hadamard
                        # overlaps the slab's m1 half
                        for mi in range(MT):
                            for dk in range(s):
                                k = k0 + dk
                                first = k == 0 and open_acc
                                last = k == p["ktiles"] - 1 and close_acc
                                lhsT = p["act"](k, mi)
                                for c in range(NC2):
                                    nc.tensor.matmul(
                                        p["psums"][mi][c][:],
                                        lhsT,
                                        w[:, dk * H1 + c * 512:
                                          dk * H1 + c * 512 + 512],
                                        start=first,
                                        stop=last,
                                        skip_group_check=not (open_acc and close_acc),
                                    )
                        p["si"] += 1

            def hadamard(early, late, tag, bufs):
                """qT[m][128, t, 128] (bf16 SBUF) = transpose(early*late).
                `early` is the psum pair whose accumulation stops first: it
                is consumed by the bounce copies (so its banks — which the
                next stage reuses — free before the late stream even ends);
                `late` is consumed by the muls. If `early` holds SBUF
                tensors ([m] list of [128, H1] tiles), the bounce copy is
                skipped and the mul reads SBUF directly."""
                qTs = []
                sbuf_early = not isinstance(early[0], list)
                for mi in range(MT):
                    q = inter.tile([128, H1], dt, tag="q", bufs=2, name="q")
                    for c in range(NC2):
                        if sbuf_early:
                            src = early[mi][:, c * 512:(c + 1) * 512]
                        else:
                            src = inter.tile([128, 512], f32, tag="bounce",
                                             bufs=2, name="bounce")
                            nc.vector.tensor_copy(src[:], early[mi][c][:])
                            src = src[:]
                        nc.vector.tensor_mul(
                            q[:, c * 512:(c + 1) * 512], late[mi][c][:], src
                        )
                    qT = inter.tile([128, (H1 // 128) * 128], dt, tag=tag,
                                    bufs=bufs, name="qT")
                    qTv = qT.rearrange("p (t b) -> p t b", b=128)
                    nc.scalar.dma_start_transpose(qTv, q[:])
                    qTs.append(qTv)
                return qTs

            out_v = out.ap().rearrange("(m p) n -> m p n", p=128)

            def finish(l3):
                """acc[m] = bf16 copy of the (already fully accumulated)
                l3 psums; one out DMA per m-tile on the scalar ring."""
                for mi in range(MT):
                    acc = inter.tile([128, H2], dt, tag="acc", bufs=2,
                                     name="acc")
                    for c in range(NC2):
                        nc.vector.tensor_copy(
                            acc[:, c * 512:(c + 1) * 512], l3[mi][c][:]
                        )
                    nc.scalar.dma_start(out_v[mi], acc[:])

            if role == "vis":
                # sync ring: weight slabs in PE need-order; scalar ring:
                # act chunks paced with the level-1 slabs.
                # cold window: tiny act chunks + ramp weight slabs first,
                # the bulk act chunks after the ramp
                order = [("act", "v1T", (0, 1)), ("act", "v2T", (0, 1)),
                         ("w", "V1", 0), ("w", "V2", 0),
                         ("act", "v1T", (1, 3)), ("act", "v2T", (1, 3)),
                         ("w", "V1", 1), ("w", "V2", 1),
                         ("w", "V1", 2), ("w", "V2", 2),
                         ("act", "v1T", (3, 8)), ("act", "v2T", (3, 8)),
                         ("w", "V1", 3), ("w", "V2", 3),
                         ("act", "v1T", (8, 32)), ("act", "v2T", (8, 32))]
                for i in range(4, len(slab_plan["V1"])):
                    order += [("w", "V1", i), ("w", "V2", i)]
                order += [("act", "mT", (0, 8)), ("act", "hT", (0, 8))]
                for nm in ("U1", "U2", "C2", "C1", "U3", "C3"):
                    for i in range(len(slab_plan[nm])):
                        order.append(("w", nm, i))
                _sched(order)

                # stage A: V level-1
                pa = alloc_ps("pa")
                pb = alloc_ps("pb")
                stream([
                    (pa, lambda k, mi: act_sb["v1T"][:, k, mi, :], "V1"),
                    (pb, lambda k, mi: act_sb["v2T"][:, k, mi, :], "V2"),
                ])
                t1T = hadamard(pb, pa, tag="qT", bufs=4)
                # stage B: U level-1 (fills PE while V hadamard runs)
                pau = alloc_ps("pau")
                pbu = alloc_ps("pbu")
                stream([
                    (pau, lambda k, mi: act_sb["hT"][:, k, mi, :], "U1"),
                    (pbu, lambda k, mi: act_sb["mT"][:, k, mi, :], "U2"),
                ])
                uT = hadamard(pbu, pau, tag="qT", bufs=4)
                # stage B2: C2 alone; its t2 result is parked in SBUF so
                # the l3 group can reuse these banks without waiting on
                # the final hadamard.
                pc2 = alloc_ps("pc2")
                stream([(pc2, lambda k, mi: act_sb["mT"][:, k, mi, :], "C2")])
                t2s = []
                for mi in range(MT):
                    t2 = inter.tile([128, H1], f32, tag="t2s", bufs=2,
                                    name="t2s")
                    for c in range(NC2):
                        nc.vector.tensor_copy(
                            t2[:, c * 512:(c + 1) * 512], pc2[mi][c][:]
                        )
                    t2s.append(t2)
                # stage C: C1 alone (t1T stationary was ready long ago)
                pc1 = alloc_ps("pc1")
                stream([(pc1, lambda k, mi: t1T[mi][:, k, :], "C1")])
                q2T = hadamard(t2s, pc1, tag="qT", bufs=4)
                # stages D+E: U3 opens the l3 accumulation, C3 closes it —
                # the final logits sum happens in PSUM, no vector add.
                l3 = alloc_ps("l3")
                stream([(l3, lambda k, mi: uT[mi][:, k, :], "U3")],
                       m_outer=True, close_acc=False)
                stream([(l3, lambda k, mi: q2T[mi][:, k, :], "C3")],
                       m_outer=True, open_acc=False)
                finish(l3)
            else:
                # cold window: mT + W2 ramp + tiny xT chunk + W1 ramp first,
                # bulk xT chunks after the ramp
                order = [("act", "mT", (0, 2)), ("w", "W2", 0),
                         ("act", "xT", (0, 1)), ("w", "W1", 0),
                         ("act", "mT", (2, 8)),
                         ("w", "W2", 1), ("w", "W1", 1),
                         ("act", "xT", (1, 3)), ("w", "W1", 2),
                         ("act", "xT", (3, 8)), ("w", "W1", 3),
                         ("act", "xT", (8, 32)), ("w", "W1", 4),
                         ("act", "xT", (32, 94))]
                for i in range(5, len(slab_plan["W1"])):
                    order.append(("w", "W1", i))
                for si in range(len(slab_plan["W3"])):
                    order.append(("w", "W3", si))
                _sched(order)

                # W2 first (small) so its psums sit in banks 0-3 while the
                # long W1 stream fills banks 4-7
                pb = alloc_ps("pb")
                pa = alloc_ps("pa")
                stream([
                    (pb, lambda k, mi: act_sb["mT"][:, k, mi, :], "W2"),
                    (pa, lambda k, mi: act_sb["xT"][:, k, mi, :], "W1"),
                ])
                xT_q = hadamard(pb, pa, tag="qT", bufs=2)
                l3w = alloc_ps("l3w")
                stream([(l3w, lambda k, mi: xT_q[mi][:, k, :], "W3")],
                       m_outer=True)
                finish(l3w)

    nc.compile()
    return nc


def _make_runner(nc, devices):
    """Adapted from concourse.bass2jax.run_bass_via_pjrt: same lowering,
    but runs on an explicit device subset and returns unmaterialized jax
    arrays so two programs can be dispatched concurrently."""
    bass2jax.install_neuronx_cc_hook()

    assert nc.dbg_addr is None
    partition_name = (
        nc.partition_id_tensor.name if nc.partition_id_tensor else None
    )

    in_names, out_names, out_avals, zero_outs = [], [], [], []
    for alloc in nc.m.functions[0].allocations:
        if not isinstance(alloc, mybir.MemoryLocationSet):
            continue
        name = alloc.memorylocations[0].name
        if alloc.kind == "ExternalInput":
            if name != partition_name:
                in_names.append(name)
        elif alloc.kind == "ExternalOutput":
            shape = tuple(alloc.tensor_shape)
            dtype = mybir.dt.np(alloc.dtype)
            out_names.append(name)
            out_avals.append(jax.core.ShapedArray(shape, dtype))
            zero_outs.append(np.zeros(shape, dtype))
    n_params = len(in_names)
    n_outs = len(out_avals)
    in_names.extend(out_names)
    if partition_name is not None:
        in_names.append(partition_name)
    donate = tuple(range(n_params, n_params + n_outs))

    def _body(*args):
        operands = list(args)
        if partition_name is not None:
            operands.append(bass2jax.partition_id_tensor())
        outs = bass2jax._bass_exec_p.bind(
            *operands,
            out_avals=tuple(out_avals),
            in_names=tuple(in_names),
            out_names=tuple(out_names),
            lowering_input_output_aliases=(),
            sim_require_finite=True,
            sim_require_nnan=True,
            nc=nc,
        )
        return tuple(outs)

    n_cores = len(devices)
    mesh = Mesh(np.asarray(devices), ("core",))
    in_specs = (PartitionSpec("core"),) * (n_params + n_outs)
    out_specs = (PartitionSpec("core"),) * n_outs
    sharded = jax.jit(
        shard_map(
            _body, mesh=mesh, in_specs=in_specs, out_specs=out_specs,
            check_rep=False,
        ),
        donate_argnums=donate,
        keep_unused=True,
    )

    def run(in_maps):
        assert len(in_maps) == n_cores
        concat_in = [
            np.concatenate(
                [np.asarray(in_maps[c][name]) for c in range(n_cores)], axis=0
            )
            for name in in_names[:n_params]
        ]
        concat_zeros = [
            np.zeros((n_cores * z.shape[0], *z.shape[1:]), z.dtype)
            for z in zero_outs
        ]
        out_arrs = sharded(*concat_in, *concat_zeros)
        return out_names, out_avals, out_arrs

    return run


def _tile_actT(a, kdim):
    """[256 batch, K<=kdim] -> SBUF image [128, (kdim/128) * 256]:
    (p, (t*2+mi)*128+b) = a[mi*128+b, t*128+p], contiguous per partition."""
    ktiles = kdim // 128
    a = np.asarray(a, np.float32)
    if a.shape[1] < kdim:
        a = np.pad(a, ((0, 0), (0, kdim - a.shape[1])))
    # [2m, 128b, ktiles, 128p] -> [128p, ktiles, 2m, 128b]
    r = a.reshape(MT, 128, ktiles, 128).transpose(3, 2, 0, 1)
    return np.ascontiguousarray(r.reshape(128, ktiles * B), dtype=_np_dt())


def _w_img(w, kdim, fp8=False, prescale=1.0):
    """[K<=kdim, H1] -> k-tile-major SBUF image [128, (kdim/128)*H1]:
    (p, t*H1 + n) = w[t*128+p, n], long-contiguous per partition.
    fp8: scale into e3m4's normal band and emit 1-byte elements."""
    kt = kdim // 128
    a = np.asarray(w, np.float32) * prescale
    if a.shape[0] < kdim:
        a = np.pad(a, ((0, kdim - a.shape[0]), (0, 0)))
    img = a.reshape(kt, 128, H1).transpose(1, 0, 2).reshape(128, kt * H1)
    if fp8:
        img = np.clip(img * W8_SCALE, -W8_MAX, W8_MAX)
        return np.ascontiguousarray(img).astype(
            mybir.dt.np(mybir.dt.float8e3))
    return np.ascontiguousarray(img, dtype=_np_dt())


def kernel(prev_h, prev_c, x, m, v1, v2, V1, V2, C1, C2, C3, W1, W2, W3, U1, U2, U3, b):
    if "runners" not in _cache:
        devs = jax.devices()
        nc_vis = build_program("vis")
        nc_inp = build_program("inp")
        _cache["runners"] = (
            _make_runner(nc_vis, devs[0:4]),
            _make_runner(nc_inp, devs[4:8]),
        )
        _cache["ncs"] = (nc_vis, nc_inp)
    run_vis, run_inp = _cache["runners"]

    v1T_img = _tile_actT(v1, V)
    v2T_img = _tile_actT(v2, V)
    mT_img = _tile_actT(m, MM)
    hT_img = _tile_actT(prev_h, H2)
    xT_img = _tile_actT(x, XP)

    vis_maps, inp_maps = [], []
    for g in range(G):
        vis_maps.append({
            "v1T": v1T_img, "v2T": v2T_img, "mT": mT_img, "hT": hT_img,
            "V1": _w_img(V1[g], V, fp8=True),
            "V2": _w_img(V2[g], V, fp8=True),
            # t1 arrives scaled by W8_SCALE^2; fold the inverse into C1
            "C1": _w_img(C1[g], VH, prescale=1.0 / (W8_SCALE * W8_SCALE)),
            "C2": _w_img(C2[g], MM),
            "C3": _w_img(C3[g], H1),
            "U1": _w_img(U1[g], H2),
            "U2": _w_img(U2[g], MM),
            "U3": _w_img(U3[g], H1),
        })
        inp_maps.append({
            "xT": xT_img, "mT": mT_img,
            "W1": _w_img(W1[g], XP, fp8=True),
            "W2": _w_img(W2[g], MM),
            # t3 arrives scaled by W8_SCALE; fold the inverse into W3
            "W3": _w_img(W3[g], H1, prescale=1.0 / W8_SCALE),
        })

    _cache["last_in_maps"] = (vis_maps, inp_maps)

    # dispatch both programs; they run concurrently on disjoint cores
    vnames, vavals, vouts = run_vis(vis_maps)
    inames, iavals, iouts = run_inp(inp_maps)

    vis_out = np.asarray(vouts[0]).astype(np.float32).reshape(G, B, H2)
    inp_out = np.asarray(iouts[0]).astype(np.float32).reshape(G, B, H2)

    logits = vis_out + inp_out + np.asarray(b, np.float32)[:, None, :]

    def sigmoid(z):
        return 1.0 / (1.0 + np.exp(-z))

    i = sigmoid(logits[0])
    f = sigmoid(logits[1])
    o = sigmoid(logits[2])
    cg = np.tanh(logits[3])
    prev_c = np.asarray(prev_c, np.float32)
    new_c = f * prev_c + i * cg
    new_h = o * np.tanh(prev_c)
    return new_h.astype(np.float32), new_c.astype(np.float32)


# revision 35
# speedup vs baseline: 1.0316x; 1.0316x over previous
"""DenseCaptioner LSTM-gate kernel for 8 Trainium2 NeuronCores.

Role-split sharding (halves per-core HBM traffic vs. gate+batch-half
data parallelism):
  cores 0-3  run program VIS: visual + recurrent paths for gate g = core,
             full batch (two 128-row m-tiles)  -> partial logits [256,1024]
  cores 4-7  run program INP: input path for gate g = core-4, full batch
             -> partial logits [256,1024]
Host: logits[g] = vis_part[g] + inp_part[g] + b[g], then sigmoid/tanh gate
math and the prev_c recurrence.

Perf structure:
  - acts stationary (one [128,128] LDWEIGHTS per (k-tile, m-tile) covers
    1024 streamed weight columns), weights moving in 512-col chunks
  - level-1 weights (V1, V2, W1) ship as FP8_EXP3/e3m4: half the DMA
    bytes of the dominant streams at unchanged PE speed (fp8 without
    DoubleRow streams at bf16 rate; mixed bf16-stationary x fp8-moving
    is numerically exact in the PE's e6m3/e10m23 pipeline).  Weights are
    pre-scaled by 128 into e3m4's [0.25, 15.5] normal band; the inverse
    scale folds into the bf16 C1/W3 images host-side.  Rel err 1.57e-2
    vs the 2e-2 gate (all-bf16: 4.5e-3; deterministic inputs).
  - everything else bf16
  - weights shipped in k-tile-major SBUF-image layout [128, ktiles*H1]
    so every weight DMA is long-contiguous per partition; streamed in
    slabs with a [1, 2, 5] ramp so the PE starts ~3us after the Tile
    preamble; 6 warm-up matmuls on a memset tile fill the preamble->
    first-slab window and ramp the PE to max pstate for free
  - ring split: ALL weight slabs go on the sync (SP) HWDGE ring;
    hadamard transposes + output DMAs go on the scalar (ACT) ring; act
    chunks go through the gpsimd SWDGE path.  Tile tracks HWDGE
    completions on 8 shared semaphore lanes assigned round-robin over
    BOTH HWDGE rings, so every HWDGE DMA orders behind the 8th-previous
    one; keeping acts off those lanes makes the weight window
    self-paced and leaves the critical-path transposes ordered behind
    early slabs only.
  - PSUM allocated in [128, 512] single-bank chunks, one tag rotating
    all 8 banks; hadamard consumes chunk-wise (bounce eats the stream
    that stops first, freeing the next stage's banks early)
  - vis tail restructured: C2 runs alone right after U1U2 and its t2
    psums are copied to SBUF (freeing the banks the l3 group reuses
    without waiting on the final hadamard); C1 runs alone against the
    SBUF-resident t2 (no bounce copy); U3 opens the l3 accumulation
    (start=True, no stop), C3 continues and closes it -> the logits sum
    happens in PSUM, no vector add in the tail
  - Hadamard transposes moved off the PE onto the DMA XBAR
    (dma_start_transpose: out[p, t, b] = in[b, t*128+p], verified)
  - vis interleaves the V-chain and U-chain so the PE never waits on
    vector/XBAR between levels; last stages run m-outer so the first
    m-tile's output DMA overlaps the second's matmuls

The two programs are dispatched concurrently on disjoint device subsets
through a copy of concourse's PJRT runner that takes an explicit device
list (the stock one hardcodes jax.devices()[:n]).
"""

import numpy as np

import jax
from jax.experimental.shard_map import shard_map
from jax.sharding import Mesh, PartitionSpec

import concourse.mybir as mybir
import concourse.tile as tile
from concourse import bacc, bass2jax

B, X, V, MM, VH, H1, H2, G = 256, 12000, 4096, 1024, 1024, 1024, 1024, 4
XP = 12032  # X padded to a multiple of 128 (94 k-tiles)
N_CORES = 8
MT = 2      # m-tiles (batch 256 = 2 x 128)
NC2 = 2     # 512-col chunks per 1024 row (one PSUM bank each)

DT_NAME = "bfloat16"

# Level-1 weights ship as FP8_EXP3 (e3m4): half the DMA bytes of the
# dominant streams at unchanged PE speed (fp8 without DoubleRow runs at
# bf16 rate; only the moving operand's dtype changes).  Weights are
# pre-scaled by W8_SCALE into e3m4's [0.25, 15.5] normal band; the
# inverse scale is folded into the next-level bf16 weights (C1, W3) on
# the host.  End-to-end rel err ~1.6e-2 vs the 2e-2 gate (bf16: 4.5e-3).
W8_NAMES = {"V1", "V2", "W1"}
W8_SCALE = 128.0
W8_MAX = 15.5

_cache = {}


def _mm_dt():
    return getattr(mybir.dt, DT_NAME)


def _np_dt():
    return mybir.dt.np(_mm_dt())


def _slab_sizes(kt, ramp=False):
    """Slab progression. ramp=True starts [1, 2, 5] so the PE's first
    matmul only waits on a 256KB transfer; later-stage weights use full
    8-k-tile slabs (bigger transfers = better per-queue DMA efficiency)."""
    sizes = []
    rem = kt
    if ramp:
        for s in (1, 2, 5):
            s = min(s, rem)
            if s:
                sizes.append(s)
            rem -= s
    while rem > 0:
        sizes.append(min(8, rem))
        rem -= 8
    return sizes


def build_program(role):
    """role "vis": visual+recurrent paths; "inp": input path. Full batch."""
    dt = _mm_dt()
    f32 = mybir.dt.float32

    nc = bacc.Bacc("TRN2", target_bir_lowering=False, debug=False)

    if role == "vis":
        act_specs = {"v1T": V, "v2T": V, "mT": MM, "hT": H2}
        w_specs = {"V1": V, "V2": V, "C1": VH, "C2": MM, "C3": H1,
                   "U1": H2, "U2": MM, "U3": H1}
    else:
        act_specs = {"xT": XP, "mT": MM}
        w_specs = {"W1": XP, "W2": MM, "W3": H1}

    def wdt(name):
        return mybir.dt.float8e3 if name in W8_NAMES else dt

    acts_d = {
        name: nc.dram_tensor(name, [128, k // 128 * B], dt, kind="ExternalInput")
        for name, k in act_specs.items()
    }
    # weights in k-tile-major SBUF-image layout [128, ktiles*H1]
    wt = {
        name: nc.dram_tensor(name, [128, (k // 128) * H1], wdt(name),
                             kind="ExternalInput")
        for name, k in w_specs.items()
    }
    # partial logits leave in bf16: half the output DMA, ~0.4% of the
    # logit magnitude in rounding — negligible against the fp8 error
    out = nc.dram_tensor("out", [B, H2], dt, kind="ExternalOutput")

    with tile.TileContext(nc) as tc:
        with (
            tc.tile_pool(name="acts", bufs=1) as acts,
            tc.tile_pool(name="wstream", bufs=4) as wstream,
            tc.tile_pool(name="wstream8", bufs=6) as wstream8,
            tc.tile_pool(name="inter", bufs=1) as inter,
            tc.tile_pool(name="ps", bufs=8, space="PSUM") as ps,
        ):
            # --- pre-plan bulk DMAs. Weights ride the sync (SP) HWDGE
            # ring exclusively (self-paced by wstream buffer recycling);
            # acts ride the scalar (ACT) ring, which later also carries
            # the critical-path hadamard transposes + output DMAs with
            # nothing slow queued ahead of them. ---
            act_tiles = {}
            for name in act_specs:
                ktiles = act_specs[name] // 128
                t = acts.tile([128, ktiles * B], dt, tag=name, name=name)
                act_tiles[name] = t

            ramp_w = {"V1", "V2"} if role == "vis" else {"W1"}
            slab_plan = {}  # wname -> list of (k0, s, tile)
            for name in w_specs:
                plan, k0 = [], 0
                if role == "inp" and name == "W2":
                    sizes = [2, 6]
                else:
                    sizes = _slab_sizes(w_specs[name] // 128, ramp=name in ramp_w)
                for s in sizes:
                    if name in W8_NAMES:
                        w = wstream8.tile([128, 8 * H1], mybir.dt.float8e3,
                                          tag="w8", name="w8")
                    else:
                        w = wstream.tile([128, 8 * H1], dt, tag="w", name="w")
                    plan.append((k0, s, w))
                    k0 += s
                slab_plan[name] = plan

            def _sched(order):
                """order: ("act", name, (kt0, kt1)) or ("w", name, si).
                Weight slabs alternate across the two HWDGE rings (sync/
                scalar) for 2x cold-phase throughput; acts are few and
                coarse so the shared 8-lane completion window stays
                weight-dominated.  Hadamard transposes + output DMAs are
                emitted later on the scalar ring, ordered only behind
                early-completing slabs in the lane window."""
                wq = 0
                for item in order:
                    name = item[1]
                    if item[0] == "act":
                        t0, t1 = item[2]
                        t1 = min(t1, act_specs[name] // 128)
                        if t0 >= t1:
                            continue
                        lo, hi = t0 * B, t1 * B
                        eng = nc.sync if wq % 2 == 0 else nc.scalar
                        wq += 1
                        eng.dma_start(
                            act_tiles[name][:, lo:hi], acts_d[name].ap()[:, lo:hi]
                        )
                    else:
                        k0, s, w = slab_plan[name][item[2]]
                        eng = nc.sync if wq % 2 == 0 else nc.scalar
                        wq += 1
                        eng.dma_start(
                            w[:, :s * H1],
                            wt[name].ap()[:, k0 * H1:(k0 + s) * H1],
                        )

            act_sb = {
                name: act_tiles[name].rearrange("p (t m b) -> p t m b", m=MT, b=128)
                for name in act_specs
            }

            # warm-up: the tensor engine reaches max pstate only after
            # ~3us of continuous work, and the first real matmul can't
            # start before its slab lands (~3us after the preamble).
            # Fill that window with throwaway matmuls on a memset tile.
            warm = inter.tile([128, 512], dt, tag="warm", name="warm")
            nc.gpsimd.memset(warm[:], 0.0)
            warm_ps = ps.tile([128, 512], f32, tag="bank", name="warm_ps")
            for _ in range(6):
                nc.tensor.matmul(
                    warm_ps[:], warm[:, :128], warm[:],
                    start=True, stop=True,
                )

            def alloc_ps(tag_name, nchunks=NC2):
                """[m][c] grid of single-bank [128,512] psum tiles."""
                return [
                    [ps.tile([128, 512], f32, tag="bank",
                             name=f"{tag_name}_{mi}_{c}") for c in range(nchunks)]
                    for mi in range(MT)
                ]

            def stream(jobs, m_outer=False, open_acc=True, close_acc=True):
                """jobs: list of (psums[m][c], act_fn(k, mi)->lhsT, wname).
                Matmuls only; the slab DMAs were pre-issued in need-order.
                Round-robin across jobs so paired streams finish together.
                m_outer: single job, loop m-tiles outermost so m0's psum
                accumulation completes mid-stage and its consumers overlap
                the m1 half.
                open_acc/close_acc=False: leave the PSUM accumulation group
                open across a later stream call into the same banks."""
                if m_outer:
                    (psums, act, wname), = jobs
                    ktiles = w_specs[wname] // 128
                    for mi in range(MT):
                        for k0, s, w in slab_plan[wname]:
                            for dk in range(s):
                                k = k0 + dk
                                for c in range(NC2):
                                    nc.tensor.matmul(
                                        psums[mi][c][:],
                                        act(k, mi),
                                        w[:, dk * H1 + c * 512:
                                          dk * H1 + c * 512 + 512],
                                        start=k == 0 and open_acc,
                                        stop=k == ktiles - 1 and close_acc,
                                        skip_group_check=not (open_acc and close_acc),
                                    )
                    return
                plans = []
                for psums, act, wname in jobs:
                    plans.append({
                        "psums": psums, "act": act, "wname": wname,
                        "ktiles": w_specs[wname] // 128,
                        "si": 0,
                    })
                while any(p["si"] < len(slab_plan[p["wname"]]) for p in plans):
                    for p in plans:
                        slabs = slab_plan[p["wname"]]
                        if p["si"] >= len(slabs):
                            continue
                        k0, s, w = slabs[p["si"]]
                        # m-outer within each slab: m0's stop lands s
                        # matmul-pairs before m1's, so its hadamard
                        # overlaps the slab's m1 half
                        for mi in range(MT):
                            for dk in range(s):
                                k = k0 + dk
                                first = k == 0 and open_acc
                                last = k == p["ktiles"] - 1 and close_acc
                                lhsT = p["act"](k, mi)
                                for c in range(NC2):
                                    nc.tensor.matmul(
                                        p["psums"][mi][c][:],
                                        lhsT,
                                        w[:, dk * H1 + c * 512:
                                          dk * H1 + c * 512 + 512],
                                        start=first,
                                        stop=last,
                                        skip_group_check=not (open_acc and close_acc),
                                    )
                        p["si"] += 1

            def hadamard(early, late, tag, bufs):
                """qT[m][128, t, 128] (bf16 SBUF) = transpose(early*late).
                `early` is the psum pair whose accumulation stops first: it
                is consumed by the bounce copies (so its banks — which the
                next stage reuses — free before the late stream even ends);
                `late` is consumed by the muls. If `early` holds SBUF
                tensors ([m] list of [128, H1] tiles), the bounce copy is
                skipped and the mul reads SBUF directly."""
                qTs = []
                sbuf_early = not isinstance(early[0], list)
                for mi in range(MT):
                    q = inter.tile([128, H1], dt, tag="q", bufs=2, name="q")
                    for c in range(NC2):
                        if sbuf_early:
                            src = early[mi][:, c * 512:(c + 1) * 512]
                        else:
                            src = inter.tile([128, 512], f32, tag="bounce",
                                             bufs=2, name="bounce")
                            nc.vector.tensor_copy(src[:], early[mi][c][:])
                            src = src[:]
                        nc.vector.tensor_mul(
                            q[:, c * 512:(c + 1) * 512], late[mi][c][:], src
                        )
                    qT = inter.tile([128, (H1 // 128) * 128], dt, tag=tag,
                                    bufs=bufs, name="qT")
                    qTv = qT.rearrange("p (t b) -> p t b", b=128)
                    nc.scalar.dma_start_transpose(qTv, q[:])
                    qTs.append(qTv)
                return qTs

            out_v = out.ap().rearrange("(m p) n -> m p n", p=128)

            def finish(l3):
                """acc[m] = bf16 copy of the (already fully accumulated)
                l3 psums; one out DMA per m-tile on the scalar ring."""
                for mi in range(MT):
                    acc = inter.tile([128, H2], dt, tag="acc", bufs=2,
                                     name="acc")
                    for c in range(NC2):
                        nc.vector.tensor_copy(
                            acc[:, c * 512:(c + 1) * 512], l3[mi][c][:]
                        )
                    nc.scalar.dma_start(out_v[mi], acc[:])

            if role == "vis":
                # sync ring: weight slabs in PE need-order; scalar ring:
                # act chunks paced with the level-1 slabs.
                # cold window: tiny act chunks + ramp weight slabs first,
                # the bulk act chunks after the ramp
                order = [("act", "v1T", (0, 1)), ("act", "v2T", (0, 1)),
                         ("w", "V1", 0), ("w", "V2", 0),
                         ("act", "v1T", (1, 3)), ("act", "v2T", (1, 3)),
                         ("w", "V1", 1), ("w", "V2", 1),
                         ("w", "V1", 2), ("w", "V2", 2),
                         ("act", "v1T", (3, 8)), ("act", "v2T", (3, 8)),
                         ("w", "V1", 3), ("w", "V2", 3),
                         ("act", "v1T", (8, 32)), ("act", "v2T", (8, 32))]
                for i in range(4, len(slab_plan["V1"])):
                    order += [("w", "V1", i), ("w", "V2", i)]
                order += [("act", "mT", (0, 8)), ("act", "hT", (0, 8))]
                for nm in ("U1", "U2", "C2", "C1", "U3", "C3"):
                    for i in range(len(slab_plan[nm])):
                        order.append(("w", nm, i))
                _sched(order)

                # stage A: V level-1
                pa = alloc_ps("pa")
                pb = alloc_ps("pb")
                stream([
                    (pa, lambda k, mi: act_sb["v1T"][:, k, mi, :], "V1"),
                    (pb, lambda k, mi: act_sb["v2T"][:, k, mi, :], "V2"),
                ])
                t1T = hadamard(pb, pa, tag="qT", bufs=4)
                # stage B: U level-1 (fills PE while V hadamard runs)
                pau = alloc_ps("pau")
                pbu = alloc_ps("pbu")
                stream([
                    (pau, lambda k, mi: act_sb["hT"][:, k, mi, :], "U1"),
                    (pbu, lambda k, mi: act_sb["mT"][:, k, mi, :], "U2"),
                ])
                uT = hadamard(pbu, pau, tag="qT", bufs=4)
                # stage B2: C2 alone; its t2 result is parked in SBUF so
                # the l3 group can reuse these banks without waiting on
                # the final hadamard.
                pc2 = alloc_ps("pc2")
                stream([(pc2, lambda k, mi: act_sb["mT"][:, k, mi, :], "C2")])
                t2s = []
                for mi in range(MT):
                    t2 = inter.tile([128, H1], f32, tag="t2s", bufs=2,
                                    name="t2s")
                    for c in range(NC2):
                        nc.vector.tensor_copy(
                            t2[:, c * 512:(c + 1) * 512], pc2[mi][c][:]
                        )
                    t2s.append(t2)
                # stage C: C1 alone (t1T stationary was ready long ago)
                pc1 = alloc_ps("pc1")
                stream([(pc1, lambda k, mi: t1T[mi][:, k, :], "C1")])
                q2T = hadamard(t2s, pc1, tag="qT", bufs=4)
                # stages D+E: U3 opens the l3 accumulation, C3 closes it —
                # the final logits sum happens in PSUM, no vector add.
                l3 = alloc_ps("l3")
                stream([(l3, lambda k, mi: uT[mi][:, k, :], "U3")],
                       m_outer=True, close_acc=False)
                stream([(l3, lambda k, mi: q2T[mi][:, k, :], "C3")],
                       m_outer=True, open_acc=False)
                finish(l3)
            else:
                # cold window: mT + W2 ramp + tiny xT chunk + W1 ramp first,
                # bulk xT chunks after the ramp
                order = [("act", "mT", (0, 8)), ("w", "W2", 0),
                         ("act", "xT", (0, 1)), ("w", "W1", 0),
                         ("w", "W2", 1), ("w", "W1", 1),
                         ("act", "xT", (1, 3)), ("w", "W1", 2),
                         ("act", "xT", (3, 8)), ("w", "W1", 3),
                         ("act", "xT", (8, 32)), ("w", "W1", 4),
                         ("act", "xT", (32, 94))]
                for i in range(5, len(slab_plan["W1"])):
                    order.append(("w", "W1", i))
                for si in range(len(slab_plan["W3"])):
                    order.append(("w", "W3", si))
                _sched(order)

                # W2 first (small) so its psums sit in banks 0-3 while the
                # long W1 stream fills banks 4-7
                pb = alloc_ps("pb")
                pa = alloc_ps("pa")
                stream([
                    (pb, lambda k, mi: act_sb["mT"][:, k, mi, :], "W2"),
                    (pa, lambda k, mi: act_sb["xT"][:, k, mi, :], "W1"),
                ])
                xT_q = hadamard(pb, pa, tag="qT", bufs=2)
                l3w = alloc_ps("l3w")
                stream([(l3w, lambda k, mi: xT_q[mi][:, k, :], "W3")],
                       m_outer=True)
                finish(l3w)

    nc.compile()
    return nc


def _make_runner(nc, devices):
    """Adapted from concourse.bass2jax.run_bass_via_pjrt: same lowering,
    but runs on an explicit device subset and returns unmaterialized jax
    arrays so two programs can be dispatched concurrently."""
    bass2jax.install_neuronx_cc_hook()

    assert nc.dbg_addr is None
    partition_name = (
        nc.partition_id_tensor.name if nc.partition_id_tensor else None
    )

    in_names, out_names, out_avals, zero_outs = [], [], [], []
    for alloc in nc.m.functions[0].allocations:
        if not isinstance(alloc, mybir.MemoryLocationSet):
            continue
        name = alloc.memorylocations[0].name
        if alloc.kind == "ExternalInput":
            if name != partition_name:
                in_names.append(name)
        elif alloc.kind == "ExternalOutput":
            shape = tuple(alloc.tensor_shape)
            dtype = mybir.dt.np(alloc.dtype)
            out_names.append(name)
            out_avals.append(jax.core.ShapedArray(shape, dtype))
            zero_outs.append(np.zeros(shape, dtype))
    n_params = len(in_names)
    n_outs = len(out_avals)
    in_names.extend(out_names)
    if partition_name is not None:
        in_names.append(partition_name)
    donate = tuple(range(n_params, n_params + n_outs))

    def _body(*args):
        operands = list(args)
        if partition_name is not None:
            operands.append(bass2jax.partition_id_tensor())
        outs = bass2jax._bass_exec_p.bind(
            *operands,
            out_avals=tuple(out_avals),
            in_names=tuple(in_names),
            out_names=tuple(out_names),
            lowering_input_output_aliases=(),
            sim_require_finite=True,
            sim_require_nnan=True,
            nc=nc,
        )
        return tuple(outs)

    n_cores = len(devices)
    mesh = Mesh(np.asarray(devices), ("core",))
    in_specs = (PartitionSpec("core"),) * (n_params + n_outs)
    out_specs = (PartitionSpec("core"),) * n_outs
    sharded = jax.jit(
        shard_map(
            _body, mesh=mesh, in_specs=in_specs, out_specs=out_specs,
            check_rep=False,
        ),
        donate_argnums=donate,
        keep_unused=True,
    )

    def run(in_maps):
        assert len(in_maps) == n_cores
        concat_in = [
            np.concatenate(
                [np.asarray(in_maps[c][name]) for c in range(n_cores)], axis=0
            )
            for name in in_names[:n_params]
        ]
        concat_zeros = [
            np.zeros((n_cores * z.shape[0], *z.shape[1:]), z.dtype)
            for z in zero_outs
        ]
        out_arrs = sharded(*concat_in, *concat_zeros)
        return out_names, out_avals, out_arrs

    return run


def _tile_actT(a, kdim):
    """[256 batch, K<=kdim] -> SBUF image [128, (kdim/128) * 256]:
    (p, (t*2+mi)*128+b) = a[mi*128+b, t*128+p], contiguous per partition."""
    ktiles = kdim // 128
    a = np.asarray(a, np.float32)
    if a.shape[1] < kdim:
        a = np.pad(a, ((0, 0), (0, kdim - a.shape[1])))
    # [2m, 128b, ktiles, 128p] -> [128p, ktiles, 2m, 128b]
    r = a.reshape(MT, 128, ktiles, 128).transpose(3, 2, 0, 1)
    return np.ascontiguousarray(r.reshape(128, ktiles * B), dtype=_np_dt())


def _w_img(w, kdim, fp8=False, prescale=1.0):
    """[K<=kdim, H1] -> k-tile-major SBUF image [128, (kdim/128)*H1]:
    (p, t*H1 + n) = w[t*128+p, n], long-contiguous per partition.
    fp8: scale into e3m4's normal band and emit 1-byte elements."""
    kt = kdim // 128
    a = np.asarray(w, np.float32) * prescale
    if a.shape[0] < kdim:
        a = np.pad(a, ((0, kdim - a.shape[0]), (0, 0)))
    img = a.reshape(kt, 128, H1).transpose(1, 0, 2).reshape(128, kt * H1)
    if fp8:
        img = np.clip(img * W8_SCALE, -W8_MAX, W8_MAX)
        return np.ascontiguousarray(img).astype(
            mybir.dt.np(mybir.dt.float8e3))
    return np.ascontiguousarray(img, dtype=_np_dt())


def kernel(prev_h, prev_c, x, m, v1, v2, V1, V2, C1, C2, C3, W1, W2, W3, U1, U2, U3, b):
    if "runners" not in _cache:
        devs = jax.devices()
        nc_vis = build_program("vis")
        nc_inp = build_program("inp")
        _cache["runners"] = (
            _make_runner(nc_vis, devs[0:4]),
            _make_runner(nc_inp, devs[4:8]),
        )
        _cache["ncs"] = (nc_vis, nc_inp)
    run_vis, run_inp = _cache["runners"]

    v1T_img = _tile_actT(v1, V)
    v2T_img = _tile_actT(v2, V)
    mT_img = _tile_actT(m, MM)
    hT_img = _tile_actT(prev_h, H2)
    xT_img = _tile_actT(x, XP)

    vis_maps, inp_maps = [], []
    for g in range(G):
        vis_maps.append({
            "v1T": v1T_img, "v2T": v2T_img, "mT": mT_img, "hT": hT_img,
            "V1": _w_img(V1[g], V, fp8=True),
            "V2": _w_img(V2[g], V, fp8=True),
            # t1 arrives scaled by W8_SCALE^2; fold the inverse into C1
            "C1": _w_img(C1[g], VH, prescale=1.0 / (W8_SCALE * W8_SCALE)),
            "C2": _w_img(C2[g], MM),
            "C3": _w_img(C3[g], H1),
            "U1": _w_img(U1[g], H2),
            "U2": _w_img(U2[g], MM),
            "U3": _w_img(U3[g], H1),
        })
        inp_maps.append({
            "xT": xT_img, "mT": mT_img,
            "W1": _w_img(W1[g], XP, fp8=True),
            "W2": _w_img(W2[g], MM),
            # t3 arrives scaled by W8_SCALE; fold the inverse into W3
            "W3": _w_img(W3[g], H1, prescale=1.0 / W8_SCALE),
        })

    _cache["last_in_maps"] = (vis_maps, inp_maps)

    # dispatch both programs; they run concurrently on disjoint cores
    vnames, vavals, vouts = run_vis(vis_maps)
    inames, iavals, iouts = run_inp(inp_maps)

    vis_out = np.asarray(vouts[0]).astype(np.float32).reshape(G, B, H2)
    inp_out = np.asarray(iouts[0]).astype(np.float32).reshape(G, B, H2)

    logits = vis_out + inp_out + np.asarray(b, np.float32)[:, None, :]

    def sigmoid(z):
        return 1.0 / (1.0 + np.exp(-z))

    i = sigmoid(logits[0])
    f = sigmoid(logits[1])
    o = sigmoid(logits[2])
    cg = np.tanh(logits[3])
    prev_c = np.asarray(prev_c, np.float32)
    new_c = f * prev_c + i * cg
    new_h = o * np.tanh(prev_c)
    return new_h.astype(np.float32), new_c.astype(np.float32)


# revision 36
# speedup vs baseline: 1.0391x; 1.0072x over previous
"""DenseCaptioner LSTM-gate kernel for 8 Trainium2 NeuronCores.

Role-split sharding (halves per-core HBM traffic vs. gate+batch-half
data parallelism):
  cores 0-3  run program VIS: visual + recurrent paths for gate g = core,
             full batch (two 128-row m-tiles)  -> partial logits [256,1024]
  cores 4-7  run program INP: input path for gate g = core-4, full batch
             -> partial logits [256,1024]
Host: logits[g] = vis_part[g] + inp_part[g] + b[g], then sigmoid/tanh gate
math and the prev_c recurrence.

Perf structure:
  - acts stationary (one [128,128] LDWEIGHTS per (k-tile, m-tile) covers
    1024 streamed weight columns), weights moving in 512-col chunks
  - level-1 weights (V1, V2, W1) ship as FP8_EXP3/e3m4: half the DMA
    bytes of the dominant streams at unchanged PE speed (fp8 without
    DoubleRow streams at bf16 rate; mixed bf16-stationary x fp8-moving
    is numerically exact in the PE's e6m3/e10m23 pipeline).  Weights are
    pre-scaled by 128 into e3m4's [0.25, 15.5] normal band; the inverse
    scale folds into the bf16 C1/W3 images host-side.  Rel err 1.57e-2
    vs the 2e-2 gate (all-bf16: 4.5e-3; deterministic inputs).
  - everything else bf16
  - weights shipped in k-tile-major SBUF-image layout [128, ktiles*H1]
    so every weight DMA is long-contiguous per partition; streamed in
    slabs with a [1, 2, 5] ramp so the PE starts ~3us after the Tile
    preamble; 6 warm-up matmuls on a memset tile fill the preamble->
    first-slab window and ramp the PE to max pstate for free
  - ring split: ALL weight slabs go on the sync (SP) HWDGE ring;
    hadamard transposes + output DMAs go on the scalar (ACT) ring; act
    chunks go through the gpsimd SWDGE path.  Tile tracks HWDGE
    completions on 8 shared semaphore lanes assigned round-robin over
    BOTH HWDGE rings, so every HWDGE DMA orders behind the 8th-previous
    one; keeping acts off those lanes makes the weight window
    self-paced and leaves the critical-path transposes ordered behind
    early slabs only.
  - PSUM allocated in [128, 512] single-bank chunks, one tag rotating
    all 8 banks; hadamard consumes chunk-wise (bounce eats the stream
    that stops first, freeing the next stage's banks early)
  - vis tail restructured: C2 runs alone right after U1U2 and its t2
    psums are copied to SBUF (freeing the banks the l3 group reuses
    without waiting on the final hadamard); C1 runs alone against the
    SBUF-resident t2 (no bounce copy); U3 opens the l3 accumulation
    (start=True, no stop), C3 continues and closes it -> the logits sum
    happens in PSUM, no vector add in the tail
  - Hadamard transposes moved off the PE onto the DMA XBAR
    (dma_start_transpose: out[p, t, b] = in[b, t*128+p], verified)
  - vis interleaves the V-chain and U-chain so the PE never waits on
    vector/XBAR between levels; last stages run m-outer so the first
    m-tile's output DMA overlaps the second's matmuls

The two programs are dispatched concurrently on disjoint device subsets
through a copy of concourse's PJRT runner that takes an explicit device
list (the stock one hardcodes jax.devices()[:n]).
"""

import numpy as np

import jax
from jax.experimental.shard_map import shard_map
from jax.sharding import Mesh, PartitionSpec

import concourse.mybir as mybir
import concourse.tile as tile
from concourse import bacc, bass2jax

B, X, V, MM, VH, H1, H2, G = 256, 12000, 4096, 1024, 1024, 1024, 1024, 4
XP = 12032  # X padded to a multiple of 128 (94 k-tiles)
N_CORES = 8
MT = 2      # m-tiles (batch 256 = 2 x 128)
NC2 = 2     # 512-col chunks per 1024 row (one PSUM bank each)

DT_NAME = "bfloat16"

# Level-1 weights ship as FP8_EXP3 (e3m4): half the DMA bytes of the
# dominant streams at unchanged PE speed (fp8 without DoubleRow runs at
# bf16 rate; only the moving operand's dtype changes).  Weights are
# pre-scaled by W8_SCALE into e3m4's [0.25, 15.5] normal band; the
# inverse scale is folded into the next-level bf16 weights (C1, W3) on
# the host.  End-to-end rel err ~1.6e-2 vs the 2e-2 gate (bf16: 4.5e-3).
W8_NAMES = {"V1", "V2", "W1"}
W8_SCALE = 128.0
W8_MAX = 15.5

_cache = {}


def _mm_dt():
    return getattr(mybir.dt, DT_NAME)


def _np_dt():
    return mybir.dt.np(_mm_dt())


def _slab_sizes(kt, ramp=False):
    """Slab progression. ramp=True starts [1, 2, 5] so the PE's first
    matmul only waits on a 256KB transfer; later-stage weights use full
    8-k-tile slabs (bigger transfers = better per-queue DMA efficiency)."""
    sizes = []
    rem = kt
    if ramp:
        for s in (1, 2, 5):
            s = min(s, rem)
            if s:
                sizes.append(s)
            rem -= s
    while rem > 0:
        sizes.append(min(8, rem))
        rem -= 8
    return sizes


def build_program(role):
    """role "vis": visual+recurrent paths; "inp": input path. Full batch."""
    dt = _mm_dt()
    f32 = mybir.dt.float32

    nc = bacc.Bacc("TRN2", target_bir_lowering=False, debug=False)

    if role == "vis":
        act_specs = {"v1T": V, "v2T": V, "mT": MM, "hT": H2}
        w_specs = {"V1": V, "V2": V, "C1": VH, "C2": MM, "C3": H1,
                   "U1": H2, "U2": MM, "U3": H1}
    else:
        act_specs = {"xT": XP, "mT": MM}
        w_specs = {"W1": XP, "W2": MM, "W3": H1}

    def wdt(name):
        return mybir.dt.float8e3 if name in W8_NAMES else dt

    acts_d = {
        name: nc.dram_tensor(name, [128, k // 128 * B], dt, kind="ExternalInput")
        for name, k in act_specs.items()
    }
    # weights in k-tile-major SBUF-image layout [128, ktiles*H1]
    wt = {
        name: nc.dram_tensor(name, [128, (k // 128) * H1], wdt(name),
                             kind="ExternalInput")
        for name, k in w_specs.items()
    }
    # partial logits leave in bf16: half the output DMA, ~0.4% of the
    # logit magnitude in rounding — negligible against the fp8 error
    out = nc.dram_tensor("out", [B, H2], dt, kind="ExternalOutput")

    with tile.TileContext(nc) as tc:
        with (
            tc.tile_pool(name="acts", bufs=1) as acts,
            tc.tile_pool(name="wstream", bufs=4) as wstream,
            tc.tile_pool(name="wstream8", bufs=6) as wstream8,
            tc.tile_pool(name="inter", bufs=1) as inter,
            tc.tile_pool(name="ps", bufs=8, space="PSUM") as ps,
        ):
            # --- pre-plan bulk DMAs. Weights ride the sync (SP) HWDGE
            # ring exclusively (self-paced by wstream buffer recycling);
            # acts ride the scalar (ACT) ring, which later also carries
            # the critical-path hadamard transposes + output DMAs with
            # nothing slow queued ahead of them. ---
            act_tiles = {}
            for name in act_specs:
                ktiles = act_specs[name] // 128
                t = acts.tile([128, ktiles * B], dt, tag=name, name=name)
                act_tiles[name] = t

            ramp_w = {"V1", "V2"} if role == "vis" else {"W1"}
            slab_plan = {}  # wname -> list of (k0, s, tile)
            for name in w_specs:
                plan, k0 = [], 0
                if role == "inp" and name == "W2":
                    sizes = [2, 6]
                else:
                    sizes = _slab_sizes(w_specs[name] // 128, ramp=name in ramp_w)
                for s in sizes:
                    if name in W8_NAMES:
                        w = wstream8.tile([128, 8 * H1], mybir.dt.float8e3,
                                          tag="w8", name="w8")
                    else:
                        w = wstream.tile([128, 8 * H1], dt, tag="w", name="w")
                    plan.append((k0, s, w))
                    k0 += s
                slab_plan[name] = plan

            def _sched(order):
                """order: ("act", name, (kt0, kt1)) or ("w", name, si).
                Weight slabs alternate across the two HWDGE rings (sync/
                scalar) for 2x cold-phase throughput; acts are few and
                coarse so the shared 8-lane completion window stays
                weight-dominated.  Hadamard transposes + output DMAs are
                emitted later on the scalar ring, ordered only behind
                early-completing slabs in the lane window."""
                wq = 0
                for item in order:
                    name = item[1]
                    if item[0] == "act":
                        t0, t1 = item[2]
                        t1 = min(t1, act_specs[name] // 128)
                        if t0 >= t1:
                            continue
                        lo, hi = t0 * B, t1 * B
                        eng = nc.sync if wq % 2 == 0 else nc.scalar
                        wq += 1
                        eng.dma_start(
                            act_tiles[name][:, lo:hi], acts_d[name].ap()[:, lo:hi]
                        )
                    else:
                        k0, s, w = slab_plan[name][item[2]]
                        eng = nc.sync if wq % 2 == 0 else nc.scalar
                        wq += 1
                        eng.dma_start(
                            w[:, :s * H1],
                            wt[name].ap()[:, k0 * H1:(k0 + s) * H1],
                        )

            act_sb = {
                name: act_tiles[name].rearrange("p (t m b) -> p t m b", m=MT, b=128)
                for name in act_specs
            }

            # warm-up: the tensor engine reaches max pstate only after
            # ~3us of continuous work, and the first real matmul can't
            # start before its slab lands (~3us after the preamble).
            # Fill that window with throwaway matmuls on a memset tile.
            warm = inter.tile([128, 512], dt, tag="warm", name="warm")
            nc.gpsimd.memset(warm[:], 0.0)
            warm_ps = ps.tile([128, 512], f32, tag="bank", name="warm_ps")
            for _ in range(6):
                nc.tensor.matmul(
                    warm_ps[:], warm[:, :128], warm[:],
                    start=True, stop=True,
                )

            def alloc_ps(tag_name, nchunks=NC2):
                """[m][c] grid of single-bank [128,512] psum tiles."""
                return [
                    [ps.tile([128, 512], f32, tag="bank",
                             name=f"{tag_name}_{mi}_{c}") for c in range(nchunks)]
                    for mi in range(MT)
                ]

            def stream(jobs, m_outer=False, open_acc=True, close_acc=True):
                """jobs: list of (psums[m][c], act_fn(k, mi)->lhsT, wname).
                Matmuls only; the slab DMAs were pre-issued in need-order.
                Round-robin across jobs so paired streams finish together.
                m_outer: single job, loop m-tiles outermost so m0's psum
                accumulation completes mid-stage and its consumers overlap
                the m1 half.
                open_acc/close_acc=False: leave the PSUM accumulation group
                open across a later stream call into the same banks."""
                if m_outer:
                    (psums, act, wname), = jobs
                    ktiles = w_specs[wname] // 128
                    for mi in range(MT):
                        for k0, s, w in slab_plan[wname]:
                            for dk in range(s):
                                k = k0 + dk
                                for c in range(NC2):
                                    nc.tensor.matmul(
                                        psums[mi][c][:],
                                        act(k, mi),
                                        w[:, dk * H1 + c * 512:
                                          dk * H1 + c * 512 + 512],
                                        start=k == 0 and open_acc,
                                        stop=k == ktiles - 1 and close_acc,
                                        skip_group_check=not (open_acc and close_acc),
                                    )
                    return
                plans = []
                for psums, act, wname in jobs:
                    plans.append({
                        "psums": psums, "act": act, "wname": wname,
                        "ktiles": w_specs[wname] // 128,
                        "si": 0,
                    })
                while any(p["si"] < len(slab_plan[p["wname"]]) for p in plans):
                    for p in plans:
                        slabs = slab_plan[p["wname"]]
                        if p["si"] >= len(slabs):
                            continue
                        k0, s, w = slabs[p["si"]]
                        # m-outer within each slab: m0's stop lands s
                        # matmul-pairs before m1's, so its hadamard
                        # overlaps the slab's m1 half
                        for mi in range(MT):
                            for dk in range(s):
                                k = k0 + dk
                                first = k == 0 and open_acc
                                last = k == p["ktiles"] - 1 and close_acc
                                lhsT = p["act"](k, mi)
                                for c in range(NC2):
                                    nc.tensor.matmul(
                                        p["psums"][mi][c][:],
                                        lhsT,
                                        w[:, dk * H1 + c * 512:
                                          dk * H1 + c * 512 + 512],
                                        start=first,
                                        stop=last,
                                        skip_group_check=not (open_acc and close_acc),
                                    )
                        p["si"] += 1

            def hadamard(early, late, tag, bufs):
                """qT[m][128, t, 128] (bf16 SBUF) = transpose(early*late).
                `early` is the psum pair whose accumulation stops first: it
                is consumed by the bounce copies (so its banks — which the
                next stage reuses — free before the late stream even ends);
                `late` is consumed by the muls. If `early` holds SBUF
                tensors ([m] list of [128, H1] tiles), the bounce copy is
                skipped and the mul reads SBUF directly."""
                qTs = []
                sbuf_early = not isinstance(early[0], list)
                for mi in range(MT):
                    q = inter.tile([128, H1], dt, tag="q", bufs=2, name="q")
                    for c in range(NC2):
                        if sbuf_early:
                            src = early[mi][:, c * 512:(c + 1) * 512]
                        else:
                            src = inter.tile([128, 512], f32, tag="bounce",
                                             bufs=2, name="bounce")
                            nc.vector.tensor_copy(src[:], early[mi][c][:])
                            src = src[:]
                        nc.vector.tensor_mul(
                            q[:, c * 512:(c + 1) * 512], late[mi][c][:], src
                        )
                    qT = inter.tile([128, (H1 // 128) * 128], dt, tag=tag,
                                    bufs=bufs, name="qT")
                    qTv = qT.rearrange("p (t b) -> p t b", b=128)
                    nc.scalar.dma_start_transpose(qTv, q[:])
                    qTs.append(qTv)
                return qTs

            out_v = out.ap().rearrange("(m p) n -> m p n", p=128)

            def finish(l3):
                """acc[m] = bf16 copy of the (already fully accumulated)
                l3 psums; each half DMAs out right after its copy so the
                last transfer posts as early as possible."""
                for mi in range(MT):
                    acc = inter.tile([128, H2], dt, tag="acc", bufs=2,
                                     name="acc")
                    for c in range(NC2):
                        sl = slice(c * 512, (c + 1) * 512)
                        nc.vector.tensor_copy(acc[:, sl], l3[mi][c][:])
                        nc.scalar.dma_start(out_v[mi][:, sl], acc[:, sl])

            if role == "vis":
                # sync ring: weight slabs in PE need-order; scalar ring:
                # act chunks paced with the level-1 slabs.
                # cold window: tiny act chunks + ramp weight slabs first,
                # the bulk act chunks after the ramp
                order = [("act", "v1T", (0, 1)), ("act", "v2T", (0, 1)),
                         ("w", "V1", 0), ("w", "V2", 0),
                         ("act", "v1T", (1, 3)), ("act", "v2T", (1, 3)),
                         ("w", "V1", 1), ("w", "V2", 1),
                         ("w", "V1", 2), ("w", "V2", 2),
                         ("act", "v1T", (3, 8)), ("act", "v2T", (3, 8)),
                         ("w", "V1", 3), ("w", "V2", 3),
                         ("act", "v1T", (8, 32)), ("act", "v2T", (8, 32))]
                for i in range(4, len(slab_plan["V1"])):
                    order += [("w", "V1", i), ("w", "V2", i)]
                order += [("act", "mT", (0, 8)), ("act", "hT", (0, 8))]
                for nm in ("U1", "U2", "C2", "C1", "U3", "C3"):
                    for i in range(len(slab_plan[nm])):
                        order.append(("w", nm, i))
                _sched(order)

                # stage A: V level-1
                pa = alloc_ps("pa")
                pb = alloc_ps("pb")
                stream([
                    (pa, lambda k, mi: act_sb["v1T"][:, k, mi, :], "V1"),
                    (pb, lambda k, mi: act_sb["v2T"][:, k, mi, :], "V2"),
                ])
                t1T = hadamard(pb, pa, tag="qT", bufs=4)
                # stage B: U level-1 (fills PE while V hadamard runs)
                pau = alloc_ps("pau")
                pbu = alloc_ps("pbu")
                stream([
                    (pau, lambda k, mi: act_sb["hT"][:, k, mi, :], "U1"),
                    (pbu, lambda k, mi: act_sb["mT"][:, k, mi, :], "U2"),
                ])
                uT = hadamard(pbu, pau, tag="qT", bufs=4)
                # stage B2: C2 alone; its t2 result is parked in SBUF so
                # the l3 group can reuse these banks without waiting on
                # the final hadamard.
                pc2 = alloc_ps("pc2")
                stream([(pc2, lambda k, mi: act_sb["mT"][:, k, mi, :], "C2")])
                t2s = []
                for mi in range(MT):
                    t2 = inter.tile([128, H1], f32, tag="t2s", bufs=2,
                                    name="t2s")
                    for c in range(NC2):
                        nc.vector.tensor_copy(
                            t2[:, c * 512:(c + 1) * 512], pc2[mi][c][:]
                        )
                    t2s.append(t2)
                # stage C: C1 alone (t1T stationary was ready long ago)
                pc1 = alloc_ps("pc1")
                stream([(pc1, lambda k, mi: t1T[mi][:, k, :], "C1")])
                q2T = hadamard(t2s, pc1, tag="qT", bufs=4)
                # stages D+E: U3 opens the l3 accumulation, C3 closes it —
                # the final logits sum happens in PSUM, no vector add.
                l3 = alloc_ps("l3")
                stream([(l3, lambda k, mi: uT[mi][:, k, :], "U3")],
                       m_outer=True, close_acc=False)
                stream([(l3, lambda k, mi: q2T[mi][:, k, :], "C3")],
                       m_outer=True, open_acc=False)
                finish(l3)
            else:
                # cold window: mT + W2 ramp + tiny xT chunk + W1 ramp first,
                # bulk xT chunks after the ramp
                order = [("act", "mT", (0, 8)), ("w", "W2", 0),
                         ("act", "xT", (0, 1)), ("w", "W1", 0),
                         ("w", "W2", 1), ("w", "W1", 1),
                         ("act", "xT", (1, 3)), ("w", "W1", 2),
                         ("act", "xT", (3, 8)), ("w", "W1", 3),
                         ("act", "xT", (8, 32)), ("w", "W1", 4),
                         ("act", "xT", (32, 94))]
                for i in range(5, len(slab_plan["W1"])):
                    order.append(("w", "W1", i))
                for si in range(len(slab_plan["W3"])):
                    order.append(("w", "W3", si))
                _sched(order)

                # W2 first (small) so its psums sit in banks 0-3 while the
                # long W1 stream fills banks 4-7
                pb = alloc_ps("pb")
                pa = alloc_ps("pa")
                stream([
                    (pb, lambda k, mi: act_sb["mT"][:, k, mi, :], "W2"),
                    (pa, lambda k, mi: act_sb["xT"][:, k, mi, :], "W1"),
                ])
                xT_q = hadamard(pb, pa, tag="qT", bufs=2)
                l3w = alloc_ps("l3w")
                stream([(l3w, lambda k, mi: xT_q[mi][:, k, :], "W3")],
                       m_outer=True)
                finish(l3w)

    nc.compile()
    return nc


def _make_runner(nc, devices):
    """Adapted from concourse.bass2jax.run_bass_via_pjrt: same lowering,
    but runs on an explicit device subset and returns unmaterialized jax
    arrays so two programs can be dispatched concurrently."""
    bass2jax.install_neuronx_cc_hook()

    assert nc.dbg_addr is None
    partition_name = (
        nc.partition_id_tensor.name if nc.partition_id_tensor else None
    )

    in_names, out_names, out_avals, zero_outs = [], [], [], []
    for alloc in nc.m.functions[0].allocations:
        if not isinstance(alloc, mybir.MemoryLocationSet):
            continue
        name = alloc.memorylocations[0].name
        if alloc.kind == "ExternalInput":
            if name != partition_name:
                in_names.append(name)
        elif alloc.kind == "ExternalOutput":
            shape = tuple(alloc.tensor_shape)
            dtype = mybir.dt.np(alloc.dtype)
            out_names.append(name)
            out_avals.append(jax.core.ShapedArray(shape, dtype))
            zero_outs.append(np.zeros(shape, dtype))
    n_params = len(in_names)
    n_outs = len(out_avals)
    in_names.extend(out_names)
    if partition_name is not None:
        in_names.append(partition_name)
    donate = tuple(range(n_params, n_params + n_outs))

    def _body(*args):
        operands = list(args)
        if partition_name is not None:
            operands.append(bass2jax.partition_id_tensor())
        outs = bass2jax._bass_exec_p.bind(
            *operands,
            out_avals=tuple(out_avals),
            in_names=tuple(in_names),
            out_names=tuple(out_names),
            lowering_input_output_aliases=(),
            sim_require_finite=True,
            sim_require_nnan=True,
            nc=nc,
        )
        return tuple(outs)

    n_cores = len(devices)
    mesh = Mesh(np.asarray(devices), ("core",))
    in_specs = (PartitionSpec("core"),) * (n_params + n_outs)
    out_specs = (PartitionSpec("core"),) * n_outs
    sharded = jax.jit(
        shard_map(
            _body, mesh=mesh, in_specs=in_specs, out_specs=out_specs,
            check_rep=False,
        ),
        donate_argnums=donate,
        keep_unused=True,
    )

    def run(in_maps):
        assert len(in_maps) == n_cores
        concat_in = [
            np.concatenate(
                [np.asarray(in_maps[c][name]) for c in range(n_cores)], axis=0
            )
            for name in in_names[:n_params]
        ]
        concat_zeros = [
            np.zeros((n_cores * z.shape[0], *z.shape[1:]), z.dtype)
            for z in zero_outs
        ]
        out_arrs = sharded(*concat_in, *concat_zeros)
        return out_names, out_avals, out_arrs

    return run


def _tile_actT(a, kdim):
    """[256 batch, K<=kdim] -> SBUF image [128, (kdim/128) * 256]:
    (p, (t*2+mi)*128+b) = a[mi*128+b, t*128+p], contiguous per partition."""
    ktiles = kdim // 128
    a = np.asarray(a, np.float32)
    if a.shape[1] < kdim:
        a = np.pad(a, ((0, 0), (0, kdim - a.shape[1])))
    # [2m, 128b, ktiles, 128p] -> [128p, ktiles, 2m, 128b]
    r = a.reshape(MT, 128, ktiles, 128).transpose(3, 2, 0, 1)
    return np.ascontiguousarray(r.reshape(128, ktiles * B), dtype=_np_dt())


def _w_img(w, kdim, fp8=False, prescale=1.0):
    """[K<=kdim, H1] -> k-tile-major SBUF image [128, (kdim/128)*H1]:
    (p, t*H1 + n) = w[t*128+p, n], long-contiguous per partition.
    fp8: scale into e3m4's normal band and emit 1-byte elements."""
    kt = kdim // 128
    a = np.asarray(w, np.float32) * prescale
    if a.shape[0] < kdim:
        a = np.pad(a, ((0, kdim - a.shape[0]), (0, 0)))
    img = a.reshape(kt, 128, H1).transpose(1, 0, 2).reshape(128, kt * H1)
    if fp8:
        img = np.clip(img * W8_SCALE, -W8_MAX, W8_MAX)
        return np.ascontiguousarray(img).astype(
            mybir.dt.np(mybir.dt.float8e3))
    return np.ascontiguousarray(img, dtype=_np_dt())


def kernel(prev_h, prev_c, x, m, v1, v2, V1, V2, C1, C2, C3, W1, W2, W3, U1, U2, U3, b):
    if "runners" not in _cache:
        devs = jax.devices()
        nc_vis = build_program("vis")
        nc_inp = build_program("inp")
        _cache["runners"] = (
            _make_runner(nc_vis, devs[0:4]),
            _make_runner(nc_inp, devs[4:8]),
        )
        _cache["ncs"] = (nc_vis, nc_inp)
    run_vis, run_inp = _cache["runners"]

    v1T_img = _tile_actT(v1, V)
    v2T_img = _tile_actT(v2, V)
    mT_img = _tile_actT(m, MM)
    hT_img = _tile_actT(prev_h, H2)
    xT_img = _tile_actT(x, XP)

    vis_maps, inp_maps = [], []
    for g in range(G):
        vis_maps.append({
            "v1T": v1T_img, "v2T": v2T_img, "mT": mT_img, "hT": hT_img,
            "V1": _w_img(V1[g], V, fp8=True),
            "V2": _w_img(V2[g], V, fp8=True),
            # t1 arrives scaled by W8_SCALE^2; fold the inverse into C1
            "C1": _w_img(C1[g], VH, prescale=1.0 / (W8_SCALE * W8_SCALE)),
            "C2": _w_img(C2[g], MM),
            "C3": _w_img(C3[g], H1),
            "U1": _w_img(U1[g], H2),
            "U2": _w_img(U2[g], MM),
            "U3": _w_img(U3[g], H1),
        })
        inp_maps.append({
            "xT": xT_img, "mT": mT_img,
            "W1": _w_img(W1[g], XP, fp8=True),
            "W2": _w_img(W2[g], MM),
            # t3 arrives scaled by W8_SCALE; fold the inverse into W3
            "W3": _w_img(W3[g], H1, prescale=1.0 / W8_SCALE),
        })

    _cache["last_in_maps"] = (vis_maps, inp_maps)

    # dispatch both programs; they run concurrently on disjoint cores
    vnames, vavals, vouts = run_vis(vis_maps)
    inames, iavals, iouts = run_inp(inp_maps)

    vis_out = np.asarray(vouts[0]).astype(np.float32).reshape(G, B, H2)
    inp_out = np.asarray(iouts[0]).astype(np.float32).reshape(G, B, H2)

    logits = vis_out + inp_out + np.asarray(b, np.float32)[:, None, :]

    def sigmoid(z):
        return 1.0 / (1.0 + np.exp(-z))

    i = sigmoid(logits[0])
    f = sigmoid(logits[1])
    o = sigmoid(logits[2])
    cg = np.tanh(logits[3])
    prev_c = np.asarray(prev_c, np.float32)
    new_c = f * prev_c + i * cg
    new_h = o * np.tanh(prev_c)
    return new_h.astype(np.float32), new_c.astype(np.float32)


# revision 37
# speedup vs baseline: 1.0400x; 1.0009x over previous
"""DenseCaptioner LSTM-gate kernel for 8 Trainium2 NeuronCores.

Role-split sharding (halves per-core HBM traffic vs. gate+batch-half
data parallelism):
  cores 0-3  run program VIS: visual + recurrent paths for gate g = core,
             full batch (two 128-row m-tiles)  -> partial logits [256,1024]
  cores 4-7  run program INP: input path for gate g = core-4, full batch
             -> partial logits [256,1024]
Host: logits[g] = vis_part[g] + inp_part[g] + b[g], then sigmoid/tanh gate
math and the prev_c recurrence.

Perf structure:
  - acts stationary (one [128,128] LDWEIGHTS per (k-tile, m-tile) covers
    1024 streamed weight columns), weights moving in 512-col chunks
  - level-1 weights (V1, V2, W1) ship as FP8_EXP3/e3m4: half the DMA
    bytes of the dominant streams at unchanged PE speed (fp8 without
    DoubleRow streams at bf16 rate; mixed bf16-stationary x fp8-moving
    is numerically exact in the PE's e6m3/e10m23 pipeline).  Weights are
    pre-scaled by 128 into e3m4's [0.25, 15.5] normal band; the inverse
    scale folds into the bf16 C1/W3 images host-side.  Rel err 1.57e-2
    vs the 2e-2 gate (all-bf16: 4.5e-3; deterministic inputs).
  - everything else bf16
  - weights shipped in k-tile-major SBUF-image layout [128, ktiles*H1]
    so every weight DMA is long-contiguous per partition; streamed in
    slabs with a [1, 2, 5] ramp so the PE starts ~3us after the Tile
    preamble; 6 warm-up matmuls on a memset tile fill the preamble->
    first-slab window and ramp the PE to max pstate for free
  - ring split: ALL weight slabs go on the sync (SP) HWDGE ring;
    hadamard transposes + output DMAs go on the scalar (ACT) ring; act
    chunks go through the gpsimd SWDGE path.  Tile tracks HWDGE
    completions on 8 shared semaphore lanes assigned round-robin over
    BOTH HWDGE rings, so every HWDGE DMA orders behind the 8th-previous
    one; keeping acts off those lanes makes the weight window
    self-paced and leaves the critical-path transposes ordered behind
    early slabs only.
  - PSUM allocated in [128, 512] single-bank chunks, one tag rotating
    all 8 banks; hadamard consumes chunk-wise (bounce eats the stream
    that stops first, freeing the next stage's banks early)
  - vis tail restructured: C2 runs alone right after U1U2 and its t2
    psums are copied to SBUF (freeing the banks the l3 group reuses
    without waiting on the final hadamard); C1 runs alone against the
    SBUF-resident t2 (no bounce copy); U3 opens the l3 accumulation
    (start=True, no stop), C3 continues and closes it -> the logits sum
    happens in PSUM, no vector add in the tail
  - Hadamard transposes moved off the PE onto the DMA XBAR
    (dma_start_transpose: out[p, t, b] = in[b, t*128+p], verified)
  - vis interleaves the V-chain and U-chain so the PE never waits on
    vector/XBAR between levels; last stages run m-outer so the first
    m-tile's output DMA overlaps the second's matmuls

The two programs are dispatched concurrently on disjoint device subsets
through a copy of concourse's PJRT runner that takes an explicit device
list (the stock one hardcodes jax.devices()[:n]).
"""

import numpy as np

import jax
from jax.experimental.shard_map import shard_map
from jax.sharding import Mesh, PartitionSpec

import concourse.mybir as mybir
import concourse.tile as tile
from concourse import bacc, bass2jax

B, X, V, MM, VH, H1, H2, G = 256, 12000, 4096, 1024, 1024, 1024, 1024, 4
XP = 12032  # X padded to a multiple of 128 (94 k-tiles)
N_CORES = 8
MT = 2      # m-tiles (batch 256 = 2 x 128)
NC2 = 2     # 512-col chunks per 1024 row (one PSUM bank each)

DT_NAME = "bfloat16"

# Level-1 weights ship as FP8_EXP3 (e3m4): half the DMA bytes of the
# dominant streams at unchanged PE speed (fp8 without DoubleRow runs at
# bf16 rate; only the moving operand's dtype changes).  Weights are
# pre-scaled by W8_SCALE into e3m4's [0.25, 15.5] normal band; the
# inverse scale is folded into the next-level bf16 weights (C1, W3) on
# the host.  End-to-end rel err ~1.6e-2 vs the 2e-2 gate (bf16: 4.5e-3).
W8_NAMES = {"V1", "V2", "W1"}
W8_SCALE = 128.0
W8_MAX = 15.5

_cache = {}


def _mm_dt():
    return getattr(mybir.dt, DT_NAME)


def _np_dt():
    return mybir.dt.np(_mm_dt())


def _slab_sizes(kt, ramp=False):
    """Slab progression. ramp=True starts [1, 2, 5] so the PE's first
    matmul only waits on a 256KB transfer; later-stage weights use full
    8-k-tile slabs (bigger transfers = better per-queue DMA efficiency)."""
    sizes = []
    rem = kt
    if ramp:
        for s in (1, 2, 5):
            s = min(s, rem)
            if s:
                sizes.append(s)
            rem -= s
    while rem > 0:
        sizes.append(min(8, rem))
        rem -= 8
    return sizes


def build_program(role):
    """role "vis": visual+recurrent paths; "inp": input path. Full batch."""
    dt = _mm_dt()
    f32 = mybir.dt.float32

    nc = bacc.Bacc("TRN2", target_bir_lowering=False, debug=False)

    if role == "vis":
        act_specs = {"v1T": V, "v2T": V, "mT": MM, "hT": H2}
        w_specs = {"V1": V, "V2": V, "C1": VH, "C2": MM, "C3": H1,
                   "U1": H2, "U2": MM, "U3": H1}
    else:
        act_specs = {"xT": XP, "mT": MM}
        w_specs = {"W1": XP, "W2": MM, "W3": H1}

    def wdt(name):
        return mybir.dt.float8e3 if name in W8_NAMES else dt

    acts_d = {
        name: nc.dram_tensor(name, [128, k // 128 * B], dt, kind="ExternalInput")
        for name, k in act_specs.items()
    }
    # weights in k-tile-major SBUF-image layout [128, ktiles*H1]
    wt = {
        name: nc.dram_tensor(name, [128, (k // 128) * H1], wdt(name),
                             kind="ExternalInput")
        for name, k in w_specs.items()
    }
    # partial logits leave in bf16: half the output DMA, ~0.4% of the
    # logit magnitude in rounding — negligible against the fp8 error
    out = nc.dram_tensor("out", [B, H2], dt, kind="ExternalOutput")

    with tile.TileContext(nc) as tc:
        with (
            tc.tile_pool(name="acts", bufs=1) as acts,
            tc.tile_pool(name="wstream", bufs=4) as wstream,
            tc.tile_pool(name="wstream8", bufs=6) as wstream8,
            tc.tile_pool(name="inter", bufs=1) as inter,
            tc.tile_pool(name="ps", bufs=8, space="PSUM") as ps,
        ):
            # --- pre-plan bulk DMAs. Weights ride the sync (SP) HWDGE
            # ring exclusively (self-paced by wstream buffer recycling);
            # acts ride the scalar (ACT) ring, which later also carries
            # the critical-path hadamard transposes + output DMAs with
            # nothing slow queued ahead of them. ---
            act_tiles = {}
            for name in act_specs:
                ktiles = act_specs[name] // 128
                t = acts.tile([128, ktiles * B], dt, tag=name, name=name)
                act_tiles[name] = t

            ramp_w = {"V1", "V2"} if role == "vis" else {"W1"}
            slab_plan = {}  # wname -> list of (k0, s, tile)
            for name in w_specs:
                plan, k0 = [], 0
                if role == "inp" and name == "W2":
                    sizes = [2, 6]
                else:
                    sizes = _slab_sizes(w_specs[name] // 128, ramp=name in ramp_w)
                for s in sizes:
                    if name in W8_NAMES:
                        w = wstream8.tile([128, 8 * H1], mybir.dt.float8e3,
                                          tag="w8", name="w8")
                    else:
                        w = wstream.tile([128, 8 * H1], dt, tag="w", name="w")
                    plan.append((k0, s, w))
                    k0 += s
                slab_plan[name] = plan

            def _sched(order):
                """order: ("act", name, (kt0, kt1)) or ("w", name, si).
                Weight slabs alternate across the two HWDGE rings (sync/
                scalar) for 2x cold-phase throughput; acts are few and
                coarse so the shared 8-lane completion window stays
                weight-dominated.  Hadamard transposes + output DMAs are
                emitted later on the scalar ring, ordered only behind
                early-completing slabs in the lane window."""
                wq = 0
                for item in order:
                    name = item[1]
                    if item[0] == "act":
                        t0, t1 = item[2]
                        t1 = min(t1, act_specs[name] // 128)
                        if t0 >= t1:
                            continue
                        lo, hi = t0 * B, t1 * B
                        eng = nc.sync if wq % 2 == 0 else nc.scalar
                        wq += 1
                        eng.dma_start(
                            act_tiles[name][:, lo:hi], acts_d[name].ap()[:, lo:hi]
                        )
                    else:
                        k0, s, w = slab_plan[name][item[2]]
                        eng = nc.sync if wq % 2 == 0 else nc.scalar
                        wq += 1
                        eng.dma_start(
                            w[:, :s * H1],
                            wt[name].ap()[:, k0 * H1:(k0 + s) * H1],
                        )

            act_sb = {
                name: act_tiles[name].rearrange("p (t m b) -> p t m b", m=MT, b=128)
                for name in act_specs
            }

            # warm-up: the tensor engine reaches max pstate only after
            # ~3us of continuous work, and the first real matmul can't
            # start before its slab lands (~3us after the preamble).
            # Fill that window with throwaway matmuls on a memset tile.
            warm = inter.tile([128, 512], dt, tag="warm", name="warm")
            nc.gpsimd.memset(warm[:], 0.0)
            warm_ps = ps.tile([128, 512], f32, tag="bank", name="warm_ps")
            for _ in range(6):
                nc.tensor.matmul(
                    warm_ps[:], warm[:, :128], warm[:],
                    start=True, stop=True,
                )

            def alloc_ps(tag_name, nchunks=NC2):
                """[m][c] grid of single-bank [128,512] psum tiles."""
                return [
                    [ps.tile([128, 512], f32, tag="bank",
                             name=f"{tag_name}_{mi}_{c}") for c in range(nchunks)]
                    for mi in range(MT)
                ]

            def stream(jobs, m_outer=False, open_acc=True, close_acc=True):
                """jobs: list of (psums[m][c], act_fn(k, mi)->lhsT, wname).
                Matmuls only; the slab DMAs were pre-issued in need-order.
                Round-robin across jobs so paired streams finish together.
                m_outer: single job, loop m-tiles outermost so m0's psum
                accumulation completes mid-stage and its consumers overlap
                the m1 half.
                open_acc/close_acc=False: leave the PSUM accumulation group
                open across a later stream call into the same banks."""
                if m_outer:
                    (psums, act, wname), = jobs
                    ktiles = w_specs[wname] // 128
                    for mi in range(MT):
                        for k0, s, w in slab_plan[wname]:
                            for dk in range(s):
                                k = k0 + dk
                                for c in range(NC2):
                                    nc.tensor.matmul(
                                        psums[mi][c][:],
                                        act(k, mi),
                                        w[:, dk * H1 + c * 512:
                                          dk * H1 + c * 512 + 512],
                                        start=k == 0 and open_acc,
                                        stop=k == ktiles - 1 and close_acc,
                                        skip_group_check=not (open_acc and close_acc),
                                    )
                    return
                plans = []
                for psums, act, wname in jobs:
                    plans.append({
                        "psums": psums, "act": act, "wname": wname,
                        "ktiles": w_specs[wname] // 128,
                        "si": 0,
                    })
                while any(p["si"] < len(slab_plan[p["wname"]]) for p in plans):
                    for p in plans:
                        slabs = slab_plan[p["wname"]]
                        if p["si"] >= len(slabs):
                            continue
                        k0, s, w = slabs[p["si"]]
                        # m-outer within each slab: m0's stop lands s
                        # matmul-pairs before m1's, so its hadamard
                        # overlaps the slab's m1 half
                        for mi in range(MT):
                            for dk in range(s):
                                k = k0 + dk
                                first = k == 0 and open_acc
                                last = k == p["ktiles"] - 1 and close_acc
                                lhsT = p["act"](k, mi)
                                for c in range(NC2):
                                    nc.tensor.matmul(
                                        p["psums"][mi][c][:],
                                        lhsT,
                                        w[:, dk * H1 + c * 512:
                                          dk * H1 + c * 512 + 512],
                                        start=first,
                                        stop=last,
                                        skip_group_check=not (open_acc and close_acc),
                                    )
                        p["si"] += 1

            def hadamard(early, late, tag, bufs):
                """qT[m][128, t, 128] (bf16 SBUF) = transpose(early*late).
                `early` is the psum pair whose accumulation stops first: it
                is consumed by the bounce copies (so its banks — which the
                next stage reuses — free before the late stream even ends);
                `late` is consumed by the muls. If `early` holds SBUF
                tensors ([m] list of [128, H1] tiles), the bounce copy is
                skipped and the mul reads SBUF directly."""
                qTs = []
                sbuf_early = not isinstance(early[0], list)
                for mi in range(MT):
                    q = inter.tile([128, H1], dt, tag="q", bufs=2, name="q")
                    for c in range(NC2):
                        if sbuf_early:
                            src = early[mi][:, c * 512:(c + 1) * 512]
                        else:
                            src = inter.tile([128, 512], f32, tag="bounce",
                                             bufs=2, name="bounce")
                            nc.vector.tensor_copy(src[:], early[mi][c][:])
                            src = src[:]
                        nc.vector.tensor_mul(
                            q[:, c * 512:(c + 1) * 512], late[mi][c][:], src
                        )
                    qT = inter.tile([128, (H1 // 128) * 128], dt, tag=tag,
                                    bufs=bufs, name="qT")
                    qTv = qT.rearrange("p (t b) -> p t b", b=128)
                    nc.scalar.dma_start_transpose(qTv, q[:])
                    qTs.append(qTv)
                return qTs

            out_v = out.ap().rearrange("(m p) n -> m p n", p=128)

            def finish(l3):
                """acc[m] = bf16 copy of the (already fully accumulated)
                l3 psums; each half DMAs out right after its copy so the
                last transfer posts as early as possible."""
                for mi in range(MT):
                    acc = inter.tile([128, H2], dt, tag="acc", bufs=2,
                                     name="acc")
                    for c in range(NC2):
                        sl = slice(c * 512, (c + 1) * 512)
                        nc.vector.tensor_copy(acc[:, sl], l3[mi][c][:])
                        nc.scalar.dma_start(out_v[mi][:, sl], acc[:, sl])

            if role == "vis":
                # sync ring: weight slabs in PE need-order; scalar ring:
                # act chunks paced with the level-1 slabs.
                # cold window: tiny act chunks + ramp weight slabs first,
                # the bulk act chunks after the ramp
                order = [("act", "v1T", (0, 1)), ("act", "v2T", (0, 1)),
                         ("w", "V1", 0), ("w", "V2", 0),
                         ("act", "v1T", (1, 3)), ("act", "v2T", (1, 3)),
                         ("w", "V1", 1), ("w", "V2", 1),
                         ("w", "V1", 2), ("w", "V2", 2),
                         ("act", "v1T", (3, 8)), ("act", "v2T", (3, 8)),
                         ("w", "V1", 3), ("w", "V2", 3),
                         ("act", "v1T", (8, 32)), ("act", "v2T", (8, 32))]
                for i in range(4, len(slab_plan["V1"])):
                    order += [("w", "V1", i), ("w", "V2", i)]
                order += [("act", "mT", (0, 8)), ("act", "hT", (0, 8))]
                for nm in ("U1", "U2", "C2", "C1", "U3", "C3"):
                    for i in range(len(slab_plan[nm])):
                        order.append(("w", nm, i))
                _sched(order)

                # stage A: V level-1
                pa = alloc_ps("pa")
                pb = alloc_ps("pb")
                stream([
                    (pa, lambda k, mi: act_sb["v1T"][:, k, mi, :], "V1"),
                    (pb, lambda k, mi: act_sb["v2T"][:, k, mi, :], "V2"),
                ])
                t1T = hadamard(pb, pa, tag="qT", bufs=4)
                # stage B: U level-1 (fills PE while V hadamard runs)
                pau = alloc_ps("pau")
                pbu = alloc_ps("pbu")
                stream([
                    (pau, lambda k, mi: act_sb["hT"][:, k, mi, :], "U1"),
                    (pbu, lambda k, mi: act_sb["mT"][:, k, mi, :], "U2"),
                ])
                uT = hadamard(pbu, pau, tag="qT", bufs=4)
                # stage B2: C2 alone; its t2 result is parked in SBUF so
                # the l3 group can reuse these banks without waiting on
                # the final hadamard.
                pc2 = alloc_ps("pc2")
                stream([(pc2, lambda k, mi: act_sb["mT"][:, k, mi, :], "C2")])
                t2s = []
                for mi in range(MT):
                    t2 = inter.tile([128, H1], f32, tag="t2s", bufs=2,
                                    name="t2s")
                    for c in range(NC2):
                        nc.vector.tensor_copy(
                            t2[:, c * 512:(c + 1) * 512], pc2[mi][c][:]
                        )
                    t2s.append(t2)
                # stage C: C1 alone (t1T stationary was ready long ago)
                pc1 = alloc_ps("pc1")
                stream([(pc1, lambda k, mi: t1T[mi][:, k, :], "C1")])
                q2T = hadamard(t2s, pc1, tag="qT", bufs=4)
                # stages D+E: U3 opens the l3 accumulation, C3 closes it —
                # the final logits sum happens in PSUM, no vector add.
                l3 = alloc_ps("l3")
                stream([(l3, lambda k, mi: uT[mi][:, k, :], "U3")],
                       m_outer=True, close_acc=False)
                stream([(l3, lambda k, mi: q2T[mi][:, k, :], "C3")],
                       m_outer=True, open_acc=False)
                finish(l3)
            else:
                # cold window: mT + W2 ramp + tiny xT chunk + W1 ramp first,
                # bulk xT chunks after the ramp
                order = [("act", "mT", (0, 8)), ("w", "W2", 0),
                         ("act", "xT", (0, 1)), ("w", "W1", 0),
                         ("w", "W2", 1), ("w", "W1", 1),
                         ("act", "xT", (1, 3)), ("w", "W1", 2),
                         ("act", "xT", (3, 8)), ("w", "W1", 3),
                         ("act", "xT", (8, 32)), ("w", "W1", 4),
                         ("act", "xT", (32, 94))]
                for i in range(5, len(slab_plan["W1"])):
                    order.append(("w", "W1", i))
                for si in range(len(slab_plan["W3"])):
                    order.append(("w", "W3", si))
                _sched(order)

                # W2 first (small) so its psums sit in banks 0-3 while the
                # long W1 stream fills banks 4-7
                pb = alloc_ps("pb")
                pa = alloc_ps("pa")
                stream([
                    (pb, lambda k, mi: act_sb["mT"][:, k, mi, :], "W2"),
                    (pa, lambda k, mi: act_sb["xT"][:, k, mi, :], "W1"),
                ])
                xT_q = hadamard(pb, pa, tag="qT", bufs=2)
                l3w = alloc_ps("l3w")
                stream([(l3w, lambda k, mi: xT_q[mi][:, k, :], "W3")],
                       m_outer=True)
                finish(l3w)

    nc.compile()
    return nc


def _make_runner(nc, devices):
    """Adapted from concourse.bass2jax.run_bass_via_pjrt: same lowering,
    but runs on an explicit device subset and returns unmaterialized jax
    arrays so two programs can be dispatched concurrently."""
    bass2jax.install_neuronx_cc_hook()

    assert nc.dbg_addr is None
    partition_name = (
        nc.partition_id_tensor.name if nc.partition_id_tensor else None
    )

    in_names, out_names, out_avals, zero_outs = [], [], [], []
    for alloc in nc.m.functions[0].allocations:
        if not isinstance(alloc, mybir.MemoryLocationSet):
            continue
        name = alloc.memorylocations[0].name
        if alloc.kind == "ExternalInput":
            if name != partition_name:
                in_names.append(name)
        elif alloc.kind == "ExternalOutput":
            shape = tuple(alloc.tensor_shape)
            dtype = mybir.dt.np(alloc.dtype)
            out_names.append(name)
            out_avals.append(jax.core.ShapedArray(shape, dtype))
            zero_outs.append(np.zeros(shape, dtype))
    n_params = len(in_names)
    n_outs = len(out_avals)
    in_names.extend(out_names)
    if partition_name is not None:
        in_names.append(partition_name)
    donate = tuple(range(n_params, n_params + n_outs))

    def _body(*args):
        operands = list(args)
        if partition_name is not None:
            operands.append(bass2jax.partition_id_tensor())
        outs = bass2jax._bass_exec_p.bind(
            *operands,
            out_avals=tuple(out_avals),
            in_names=tuple(in_names),
            out_names=tuple(out_names),
            lowering_input_output_aliases=(),
            sim_require_finite=True,
            sim_require_nnan=True,
            nc=nc,
        )
        return tuple(outs)

    n_cores = len(devices)
    mesh = Mesh(np.asarray(devices), ("core",))
    in_specs = (PartitionSpec("core"),) * (n_params + n_outs)
    out_specs = (PartitionSpec("core"),) * n_outs
    sharded = jax.jit(
        shard_map(
            _body, mesh=mesh, in_specs=in_specs, out_specs=out_specs,
            check_rep=False,
        ),
        donate_argnums=donate,
        keep_unused=True,
    )

    def run(in_maps):
        assert len(in_maps) == n_cores
        concat_in = [
            np.concatenate(
                [np.asarray(in_maps[c][name]) for c in range(n_cores)], axis=0
            )
            for name in in_names[:n_params]
        ]
        concat_zeros = [
            np.zeros((n_cores * z.shape[0], *z.shape[1:]), z.dtype)
            for z in zero_outs
        ]
        out_arrs = sharded(*concat_in, *concat_zeros)
        return out_names, out_avals, out_arrs

    return run


def _tile_actT(a, kdim):
    """[256 batch, K<=kdim] -> SBUF image [128, (kdim/128) * 256]:
    (p, (t*2+mi)*128+b) = a[mi*128+b, t*128+p], contiguous per partition."""
    ktiles = kdim // 128
    a = np.asarray(a, np.float32)
    if a.shape[1] < kdim:
        a = np.pad(a, ((0, 0), (0, kdim - a.shape[1])))
    # [2m, 128b, ktiles, 128p] -> [128p, ktiles, 2m, 128b]
    r = a.reshape(MT, 128, ktiles, 128).transpose(3, 2, 0, 1)
    return np.ascontiguousarray(r.reshape(128, ktiles * B), dtype=_np_dt())


def _w_img(w, kdim, fp8=False, prescale=1.0):
    """[K<=kdim, H1] -> k-tile-major SBUF image [128, (kdim/128)*H1]:
    (p, t*H1 + n) = w[t*128+p, n], long-contiguous per partition.
    fp8: scale into e3m4's normal band and emit 1-byte elements."""
    kt = kdim // 128
    a = np.asarray(w, np.float32) * prescale
    if a.shape[0] < kdim:
        a = np.pad(a, ((0, kdim - a.shape[0]), (0, 0)))
    img = a.reshape(kt, 128, H1).transpose(1, 0, 2).reshape(128, kt * H1)
    if fp8:
        img = np.clip(img * W8_SCALE, -W8_MAX, W8_MAX)
        return np.ascontiguousarray(img).astype(
            mybir.dt.np(mybir.dt.float8e3))
    return np.ascontiguousarray(img, dtype=_np_dt())


def kernel(prev_h, prev_c, x, m, v1, v2, V1, V2, C1, C2, C3, W1, W2, W3, U1, U2, U3, b):
    if "runners" not in _cache:
        devs = jax.devices()
        nc_vis = build_program("vis")
        nc_inp = build_program("inp")
        _cache["runners"] = (
            _make_runner(nc_vis, devs[0:4]),
            _make_runner(nc_inp, devs[4:8]),
        )
        _cache["ncs"] = (nc_vis, nc_inp)
    run_vis, run_inp = _cache["runners"]

    v1T_img = _tile_actT(v1, V)
    v2T_img = _tile_actT(v2, V)
    mT_img = _tile_actT(m, MM)
    hT_img = _tile_actT(prev_h, H2)
    xT_img = _tile_actT(x, XP)

    vis_maps, inp_maps = [], []
    for g in range(G):
        v_fp8 = "V1" in W8_NAMES
        w_fp8 = "W1" in W8_NAMES
        vis_maps.append({
            "v1T": v1T_img, "v2T": v2T_img, "mT": mT_img, "hT": hT_img,
            "V1": _w_img(V1[g], V, fp8=v_fp8),
            "V2": _w_img(V2[g], V, fp8=v_fp8),
            # t1 arrives scaled by W8_SCALE^2; fold the inverse into C1
            "C1": _w_img(C1[g], VH,
                         prescale=1.0 / (W8_SCALE * W8_SCALE) if v_fp8 else 1.0),
            "C2": _w_img(C2[g], MM),
            "C3": _w_img(C3[g], H1),
            "U1": _w_img(U1[g], H2),
            "U2": _w_img(U2[g], MM),
            "U3": _w_img(U3[g], H1),
        })
        inp_maps.append({
            "xT": xT_img, "mT": mT_img,
            "W1": _w_img(W1[g], XP, fp8=w_fp8),
            "W2": _w_img(W2[g], MM),
            # t3 arrives scaled by W8_SCALE; fold the inverse into W3
            "W3": _w_img(W3[g], H1,
                         prescale=1.0 / W8_SCALE if w_fp8 else 1.0),
        })

    _cache["last_in_maps"] = (vis_maps, inp_maps)

    # dispatch both programs; they run concurrently on disjoint cores
    vnames, vavals, vouts = run_vis(vis_maps)
    inames, iavals, iouts = run_inp(inp_maps)

    vis_out = np.asarray(vouts[0]).astype(np.float32).reshape(G, B, H2)
    inp_out = np.asarray(iouts[0]).astype(np.float32).reshape(G, B, H2)

    logits = vis_out + inp_out + np.asarray(b, np.float32)[:, None, :]

    def sigmoid(z):
        return 1.0 / (1.0 + np.exp(-z))

    i = sigmoid(logits[0])
    f = sigmoid(logits[1])
    o = sigmoid(logits[2])
    cg = np.tanh(logits[3])
    prev_c = np.asarray(prev_c, np.float32)
    new_c = f * prev_c + i * cg
    new_h = o * np.tanh(prev_c)
    return new_h.astype(np.float32), new_c.astype(np.float32)
